# revision 1
# baseline (speedup 1.0000x reference)
"""MoE model via vocab-precompute + bucketed one-hot selection on 8 TRN2 cores.

v2 replaced the dense W1 matmul with a per-vocab precompute (T tables) and a
per-token one-hot selection (K=512 per table).  v3 bucketed tokens by their
(i0//128, i1//128) vocab-chunk pair into 16 "pure" supertiles + n_mixed spill
supertiles, cutting selection to one matmul per table per feature chunk.

v4 removes the scalar-engine pacing found in the v3 trace:
  - b1 and bg are folded into the T/G precompute via an extra K=1 matmul row
    (ones stationary, bias as moving), so the silu evac needs no bias AP.
  - silu evacs are PAIRED: pmm tiles span 2 PSUM banks [128, 2, 512]; one
    ACTIVATE covers 1024 columns, amortizing the 352-cycle fixed cost.
  - exp FACTORIZES over the two tables: exp(G0[i0]+G1[i1]+bg) =
    expG0[i0] * expG1[i1].  exp moves into the precompute (G tables stored
    exp'd); the main loop multiplies two one-hot gate selections on the DVE.
    No Exp in the loop -> no ACT table-set switches at all.
  - the combine's gate operand is read straight from its broadcast PSUM.
"""

import os
import numpy as np
import ml_dtypes

import concourse.bass as bass
import concourse.mybir as mybir
import concourse.tile as tile
from concourse.bass_utils import run_bass_kernel_spmd

BF16 = ml_dtypes.bfloat16

B = 65536
V = 512
D = 1024
IN = 2048
E = 4
OUT = 128
NCORES = 8
BL = B // NCORES          # tokens per core
ST = 512                  # tokens per supertile (max PSUM free dim, fp32)
F = E * D                 # 4096 selected features (expert-major)
FC = F // 128             # 32 feature chunks
VC = V // 128             # 4 vocab chunks
KC = D // 128             # 8 contraction chunks per table (precompute)
FB = 4                    # W1 f-blocks streamed (1024 feats each)
FPB = 2                   # 512-wide f-pieces per f-block
NPURE = VC * VC           # 16 pure supertiles (one per chunk pair)
STM = 256                 # spill-supertile width (few tokens, cheaper MMs)

LAST_EXEC_NS = None       # set when BASSMOE_TRACE=1


def _legalize_waits(nc, max_waits=1):
    """This walrus build rejects instructions carrying more than ~1 sync-wait
    command; hoist all but the last wait onto single-wait NoOps."""
    for f in nc.m.functions:
        for bb in f.blocks:
            insts = bb.instructions
            if not any(
                inst.sync_info is not None and len(inst.sync_info.on_wait) > max_waits
                for inst in insts
            ):
                continue
            new = []
            for inst in insts:
                si = inst.sync_info
                waits = list(si.on_wait) if si is not None else []
                if len(waits) > max_waits:
                    for w in waits[:-max_waits]:
                        nop = mybir.InstNoOp(
                            name=f"legw-{nc.next_id()}", ins=[], outs=[]
                        )
                        nop.engine = inst.engine
                        nop.sync_info = mybir.SyncInfo(on_wait=[w], on_update=[])
                        new.append(nop)
                    inst.sync_info = mybir.SyncInfo(
                        on_wait=waits[-max_waits:], on_update=list(si.on_update)
                    )
                new.append(inst)
            bb.instructions = new


def _width(s):
    return ST if s < NPURE else STM


def _off(s):
    return s * ST if s <= NPURE else NPURE * ST + (s - NPURE) * STM


def _present(s):
    """Vocab chunks present in supertile s, per table."""
    if s < NPURE:
        return [[s // VC], [s % VC]]
    return [list(range(VC)), list(range(VC))]


def build_program(n_mixed, legalize=True):
    nst = NPURE + n_mixed
    dt = mybir.dt
    f32, bf16, f16 = dt.float32, dt.bfloat16, dt.float16
    AF = mybir.ActivationFunctionType
    ALU = mybir.AluOpType

    nc = bass.Bass()

    tot = NPURE * ST + n_mixed * STM
    x0d = nc.dram_tensor("x0", [1, tot], f16, kind="ExternalInput")
    x1d = nc.dram_tensor("x1", [1, tot], f16, kind="ExternalInput")
    # emb pre-transposed: embT[t, kc, p, v] = emb_t[v, kc*128+p]
    embtd = nc.dram_tensor("embt", [2, KC, 128, V], bf16, kind="ExternalInput")
    # W1 re-laid: w1m[t, fb, kc, p, ff] = W1flat[t*1024+kc*128+p, fb*1024+ff]
    w1d = nc.dram_tensor("w1m", [2, FB, KC, 128, 1024], bf16, kind="ExternalInput")
    b1rd = nc.dram_tensor("b1row", [1, F], bf16, kind="ExternalInput")
    bgrd = nc.dram_tensor("bgrow", [1, E], bf16, kind="ExternalInput")
    wgd = nc.dram_tensor("wgm", [128, 2, KC, E], bf16, kind="ExternalInput")
    w2d = nc.dram_tensor("w2s", [128, E, KC, OUT], bf16, kind="ExternalInput")
    b2d = nc.dram_tensor("b2s", [128, E], f32, kind="ExternalInput")
    seld = nc.dram_tensor("sels", [E, E, 128], bf16, kind="ExternalInput")
    ivd = nc.dram_tensor("ivs", [128, VC], f32, kind="ExternalInput")
    outd = nc.dram_tensor("out", [128, tot], f32, kind="ExternalOutput")

    with tile.TileContext(nc) as tc:
        with (
            tc.tile_pool(name="const", bufs=1) as cpool,
            tc.tile_pool(name="w1st", bufs=2) as w1pool,
            tc.tile_pool(name="xt", bufs=2) as xpool,
            tc.tile_pool(name="mask", bufs=2) as mpool,
            tc.tile_pool(name="hs", bufs=1) as hpool,
            tc.tile_pool(name="sm", bufs=2) as smpool,
            tc.tile_pool(name="gsc", bufs=1) as gspool,
            tc.tile_pool(name="accp", bufs=2) as apool,
            tc.tile_pool(name="outp", bufs=2) as opool,
            tc.tile_pool(name="pmm", bufs=2, space="PSUM") as pmm,
            tc.tile_pool(name="peo", bufs=1, space="PSUM") as peo,
            tc.tile_pool(name="pmisc", bufs=3, space="PSUM") as pmisc,
        ):
            # --- prologue loads; embt piecewise so the first MMs start early ---
            # first compute needs embt-t0 + the first W1 block: issue those
            # DMAs first, interleaved per kc, so T matmuls start at ~3us; all
            # small constants (with their ~1us fixed DMA costs) queue after.
            embt_sb = cpool.tile([128, 2, KC, V], bf16)
            wg_sb = cpool.tile([128, 2, KC, E], bf16)
            w1t00 = w1pool.tile([128, KC, 1024], bf16, tag="w1")
            b1r_sb = cpool.tile([1, F], bf16)
            for kc in range(KC):
                nc.sync.dma_start(embt_sb[:, 0, kc, :], embtd[0, kc])
                nc.sync.dma_start(w1t00[:, kc, :], w1d[0, 0, kc])
            nc.sync.dma_start(b1r_sb[:], b1rd[:])
            nc.sync.dma_start(wg_sb[:], wgd[:])
            iv_sb = cpool.tile([128, VC], f32)
            nc.sync.dma_start(iv_sb[:], ivd[:])
            ones_f16 = cpool.tile([1, 128], f16)
            nc.vector.memset(ones_f16[:], 1.0)
            ones128_bf = cpool.tile([1, 128], bf16)
            nc.vector.memset(ones128_bf[:], 1.0)
            ones4_bf = cpool.tile([E, 1], bf16)
            nc.vector.memset(ones4_bf[:], 1.0)
            bgr_sb = cpool.tile([1, E], bf16)
            nc.sync.dma_start(bgr_sb[:], bgrd[:])
            x0_pre = []
            for t, xd in enumerate((x0d, x1d)):
                xs = xpool.tile([1, ST], f16, tag=f"x{t}")
                nc.sync.dma_start(xs[:], xd[:, 0:ST])
                x0_pre.append(xs)
            t_sb = cpool.tile([128, VC, 2, FC // 4, 512], bf16)

            def emit_t_block(t, fb, w1t):
                for vc in range(VC):
                    ps = pmm.tile([128, 2, ST], f32, tag="mm")
                    for fp in range(FPB):
                        for kc in range(KC):
                            nc.tensor.matmul(
                                ps[:, fp, :],
                                embt_sb[:, t, kc, vc * 128 : (vc + 1) * 128],
                                w1t[:, kc, fp * 512 : (fp + 1) * 512],
                                start=(kc == 0),
                                stop=(t == 1 and kc == KC - 1),
                            )
                        if t == 0:
                            f0 = fb * 1024 + fp * 512
                            nc.tensor.matmul(
                                ps[:, fp, :], ones128_bf[:],
                                b1r_sb[:, f0 : f0 + 512],
                                start=False, stop=True,
                            )
                    nc.scalar.copy(
                        t_sb[:, vc, t, fb * FPB : (fb + 1) * FPB, :], ps[:]
                    )

            # block (0,0) computes while the rest of the inputs stream in
            emit_t_block(0, 0, w1t00)
            for kc in range(KC):
                nc.sync.dma_start(embt_sb[:, 1, kc, :], embtd[1, kc])
            b2_sb = cpool.tile([128, E], f32)
            nc.sync.dma_start(b2_sb[:], b2d[:])
            sel_sb = cpool.tile([E, E, 128], bf16)
            nc.sync.dma_start(sel_sb[:], seld[:])

            # --- phase 0a: exp'd gating tables (the single Exp table-set load
            #     happens here, before any Silu) ---
            g_sb = cpool.tile([128, VC, 2, E], bf16)
            for t in range(2):
                for vc in range(VC):
                    psg = pmisc.tile([128, E], f32, tag="misc")
                    for kc in range(KC):
                        nc.tensor.matmul(
                            psg[:],
                            embt_sb[:, t, kc, vc * 128 : (vc + 1) * 128],
                            wg_sb[:, t, kc, :],
                            start=(kc == 0),
                            stop=(t == 1 and kc == KC - 1),
                        )
                    if t == 0:
                        # fold bg into table 0: psg += ones(v) x bg
                        nc.tensor.matmul(
                            psg[:], ones128_bf[:], bgr_sb[:],
                            start=False, stop=True,
                        )
                    nc.scalar.activation(g_sb[:, vc, t, :], psg[:], AF.Exp, bias=0.0)

            def build_masks(i, preloaded=None):
                """x-broadcast (K=1 matmul) + one-hot compares for the chunks
                present in supertile i."""
                pres = _present(i)
                w = _width(i)
                ms = [{}, {}]
                for t, xd in enumerate((x0d, x1d)):
                    if preloaded is None:
                        xs = xpool.tile([1, w], f16, tag=f"x{t}")
                        nc.sync.dma_start(xs[:], xd[:, _off(i) : _off(i) + w])
                    else:
                        xs = preloaded[t]
                    p = pmisc.tile([128, w], f32, tag="misc")
                    nc.tensor.matmul(p[:], ones_f16[:], xs[:])
                    for vc in pres[t]:
                        m = mpool.tile([128, w], bf16, tag=f"m{t}{vc}")
                        nc.vector.tensor_scalar(
                            m[:], p[:], iv_sb[:, vc : vc + 1], None, ALU.is_equal
                        )
                        ms[t][vc] = m
                return ms

            def emit_gating(i, masks):
                """Two one-hot gate selections (exp'd tables) multiplied on the
                DVE: expt = expG0[i0] * expG1[i1]."""
                pres = _present(i)
                w = _width(i)
                sels = []
                for t in range(2):
                    sl = pmisc.tile([E, w], f32, tag="misc")
                    for j, vc in enumerate(pres[t]):
                        nc.tensor.matmul(
                            sl[:],
                            g_sb[:, vc, t, :],
                            masks[t][vc][:],
                            start=(j == 0),
                            stop=(j == len(pres[t]) - 1),
                        )
                    sels.append(sl)
                # DVE may read only one PSUM operand per op: stage sel0 in SBUF
                s0 = smpool.tile([E, w], f32, tag="s0")
                nc.vector.tensor_copy(s0[:], sels[0][:])
                expt = smpool.tile([E, w], bf16, tag="expt")
                nc.vector.tensor_tensor(expt[:], s0[:], sels[1][:], ALU.mult)
                return expt

            cur_masks = build_masks(0, preloaded=x0_pre)
            cur_expt = emit_gating(0, cur_masks)

            # --- phase 0b: remaining T blocks (block (0,0) ran above) ---
            for t in range(2):
                for fb in range(FB):
                    if t == 0 and fb == 0:
                        continue
                    w1t = w1pool.tile([128, KC, 1024], bf16, tag="w1")
                    for kc in range(KC):
                        nc.sync.dma_start(w1t[:, kc, :], w1d[t, fb, kc])
                    emit_t_block(t, fb, w1t)

            # --- remaining resident weights ---
            w2_sb = cpool.tile([128, E, KC, OUT], bf16)
            nc.sync.dma_start(w2_sb[:], w2d[:])

            for i in range(nst):
                pres = _present(i)
                w = _width(i)
                chunks = [(t, vc) for t in range(2) for vc in pres[t]]
                expt = cur_expt

                def emit_recip_chain():
                    # sum-exp -> reciprocal -> broadcast to 128 rows (PSUM).
                    sp = pmisc.tile([1, w], f32, tag="misc")
                    nc.tensor.matmul(sp[:], ones4_bf[:], expt[:])
                    rec = smpool.tile([1, w], f32, tag="rec")
                    nc.vector.reciprocal(rec[:], sp[:])
                    recb = smpool.tile([1, w], bf16, tag="recb")
                    nc.vector.tensor_copy(recb[:], rec[:])
                    rbp = pmisc.tile([128, w], f32, tag="misc")
                    nc.tensor.matmul(rbp[:], ones128_bf[:], recb[:])
                    rbs = smpool.tile([128, w], f32, tag="rbs")
                    nc.scalar.copy(rbs[:], rbp[:])
                    return rbs

                def emit_expert(e, acc):
                    # W2 for expert e (its 4 h pairs are ready) + gate-combine.
                    # gs evac on the scalar engine: the DVE FIFO must not gate
                    # the peo/pmisc PSUM rotation (head-of-line blocking).
                    eop = peo.tile([128, w], f32, tag="eo")
                    for dc in range(KC):
                        fc = e * KC + dc
                        nc.tensor.matmul(
                            eop[:],
                            w2_sb[:, e, dc, :],
                            hs[fc // 2][:, fc % 2, :],
                            start=(dc == 0),
                            stop=(dc == KC - 1),
                        )
                    gp = pmisc.tile([128, w], f32, tag="misc")
                    nc.tensor.matmul(gp[:], sel_sb[:, e, :], expt[:])
                    gs = gspool.tile([128, w], f32, tag="gs")
                    nc.scalar.copy(gs[:], gp[:])
                    if e == 0:
                        nc.vector.scalar_tensor_tensor(
                            acc[:], eop[:], b2_sb[:, e : e + 1], gs[:],
                            ALU.add, ALU.mult,
                        )
                    else:
                        tmp = opool.tile([128, w], f32, tag="outt")
                        nc.vector.scalar_tensor_tensor(
                            tmp[:], eop[:], b2_sb[:, e : e + 1], gs[:],
                            ALU.add, ALU.mult,
                        )
                        nc.vector.tensor_add(acc[:], acc[:], tmp[:])

                # --- selection + paired silu (b1 already inside T), with each
                # expert's W2+combine interleaved after its 4th silu pair ---
                next_masks = None
                hs = []
                acc = apool.tile([128, w], f32, tag="acc")
                for pair in range(FC // 2):
                    if pair == 1:
                        rbs = emit_recip_chain()
                    if pair == 7 and i + 1 < nst:
                        next_masks = build_masks(i + 1)
                    hp = pmm.tile([128, 2, ST], f32, tag="mm")
                    for half in range(2):
                        fc = pair * 2 + half
                        for j, (t, vc) in enumerate(chunks):
                            nc.tensor.matmul(
                                hp[:, half, 0:w],
                                t_sb[
                                    :, vc, t, fc // 4,
                                    (fc % 4) * 128 : (fc % 4 + 1) * 128,
                                ],
                                cur_masks[t][vc][:],
                                start=(j == 0),
                                stop=(j == len(chunks) - 1),
                            )
                    h_pair = hpool.tile([128, 2, w], bf16, tag=f"hs{pair}")
                    nc.scalar.activation(h_pair[:], hp[:, :, 0:w], AF.Silu, bias=0.0)
                    hs.append(h_pair)
                    if pair % 4 == 3:
                        emit_expert(pair // 4, acc)
                    if pair == 11 and i + 1 < nst:
                        cur_expt = emit_gating(i + 1, next_masks)

                outt = opool.tile([128, w], f32, tag="outt")
                nc.vector.tensor_tensor(outt[:], acc[:], rbs[:], ALU.mult)
                nc.sync.dma_start(outd[:, _off(i) : _off(i) + w], outt[:])
                if next_masks is not None:
                    cur_masks = next_masks

    if legalize:
        _legalize_waits(nc)
    mybir.codegen_inst_isa_subclasses(nc)
    return nc


def assign_slots(x):
    """Bucket tokens by (i0//128, i1//128) into 16 pure supertiles (512 slots,
    padded) + spill. Returns per-core slot->token maps and n_mixed."""
    x = np.asarray(x)
    slot_maps = []
    spills = []
    for c in range(NCORES):
        xc = x[c * BL : (c + 1) * BL]
        key = (xc[:, 0] // 128) * VC + xc[:, 1] // 128
        order = np.argsort(key, kind="stable")
        ks = key[order]
        slots = np.full(NPURE * ST, -1, dtype=np.int64)
        spill = []
        for b in range(NPURE):
            toks = order[ks == b]
            n = min(len(toks), ST)
            slots[b * ST : b * ST + n] = toks[:n]
            spill.extend(toks[ST:])
        slot_maps.append(slots)
        spills.append(np.array(spill, dtype=np.int64))
    n_mixed = max(
        (len(s) + STM - 1) // STM if len(s) else 0 for s in spills
    )
    full_maps = []
    for c in range(NCORES):
        m = np.full(NPURE * ST + n_mixed * STM, -1, dtype=np.int64)
        m[: NPURE * ST] = slot_maps[c]
        m[NPURE * ST : NPURE * ST + len(spills[c])] = spills[c]
        full_maps.append(m)
    return full_maps, n_mixed


def marshal_inputs(x, emb0, emb1, W1, b1, W2, b2, Wg, bg, slot_maps, n_mixed):
    """Host-side: cast/reshape full inputs into per-core in_maps."""
    x = np.asarray(x)

    xh = {"x0": [], "x1": []}
    for c in range(NCORES):
        m = slot_maps[c]
        xc = x[c * BL : (c + 1) * BL]
        xv = np.zeros((len(m), 2), dtype=np.float16)
        valid = m >= 0
        xv[valid] = xc[m[valid]].astype(np.float16)
        # pad slots: -1 matches no iv entry -> zero one-hot -> output junk
        # that the host discards.
        xv[~valid] = -1.0
        xh["x0"].append(np.ascontiguousarray(xv[:, 0].reshape(1, len(m))))
        xh["x1"].append(np.ascontiguousarray(xv[:, 1].reshape(1, len(m))))

    shared = {}
    # embT[t, kc, p, v] = emb_t[v, kc*128 + p]
    embt = np.stack(
        [np.asarray(e).T.reshape(KC, 128, V) for e in (emb0, emb1)], axis=0
    )
    shared["embt"] = np.ascontiguousarray(embt.astype(BF16))
    # W1flat[k, f] with f = e*1024 + d
    w1flat = np.asarray(W1).transpose(1, 0, 2).reshape(IN, F)
    shared["w1m"] = np.ascontiguousarray(
        w1flat.reshape(2, KC, 128, FB, 1024).transpose(0, 3, 1, 2, 4).astype(BF16)
    )
    shared["b1row"] = np.ascontiguousarray(
        np.asarray(b1).reshape(1, F).astype(BF16)
    )
    shared["bgrow"] = np.ascontiguousarray(
        np.asarray(bg).reshape(1, E).astype(BF16)
    )
    shared["wgm"] = np.ascontiguousarray(
        np.asarray(Wg).reshape(2, KC, 128, E).transpose(2, 0, 1, 3).astype(BF16)
    )
    shared["w2s"] = np.ascontiguousarray(
        np.asarray(W2).reshape(E, KC, 128, OUT).transpose(2, 0, 1, 3).astype(BF16)
    )
    shared["b2s"] = np.ascontiguousarray(np.asarray(b2).T.astype(np.float32))
    shared["sels"] = np.ascontiguousarray(
        np.broadcast_to(np.eye(E, dtype=np.float32)[:, :, None], (E, E, 128)).astype(
            BF16
        )
    )
    shared["ivs"] = np.ascontiguousarray(
        (np.arange(VC)[None, :] * 128 + np.arange(128)[:, None]).astype(np.float32)
    )
    return [
        {**{k: v[c] for k, v in xh.items()}, **shared} for c in range(NCORES)
    ]


def kernel(x, emb0, emb1, W1, b1, W2, b2, Wg, bg):
    global LAST_EXEC_NS
    slot_maps, n_mixed = assign_slots(x)
    nc = build_program(n_mixed)
    in_maps = marshal_inputs(
        x, emb0, emb1, W1, b1, W2, b2, Wg, bg, slot_maps, n_mixed
    )
    trace = os.environ.get("BASSMOE_TRACE", "0") == "1"
    res = run_bass_kernel_spmd(nc, in_maps, list(range(NCORES)), trace=trace)
    LAST_EXEC_NS = res.exec_time_ns
    out = np.empty((B, OUT), dtype=np.float32)
    for c in range(NCORES):
        m = slot_maps[c]
        valid = m >= 0
        r = res.results[c]["out"]  # [128, nst*ST]
        out[c * BL + m[valid], :] = r[:, valid].T
    return out



# revision 4
# speedup vs baseline: 1.2661x; 1.2661x over previous
"""MoE model via global vocab-pair bucketing + per-core chunk tables on 8 TRN2
cores.

v5 reworks v4's per-core bucketing into a GLOBAL (i0//128, i1//128) pair
bucketing: the host assigns each of the 16 chunk-pair classes to a core (2 per
core, sharing the i0 chunk), so each core precomputes only the 3 vocab-chunk
tables its tokens can touch (T0[c0], T1[c1a], T1[c1b]) instead of all 8 —
cutting the T = emb @ W1 precompute from 278k to ~104k PE cycles — and every
supertile is pure (2 selection matmuls per feature chunk, no mixed spill
tiles).

The main loop is EXPERT-PHASE-MAJOR: phase fb streams W1 block fb (2 x 2 MB),
builds the fb-slice of the 3 chunk tables, then for every supertile does the
8-fc selection + paired silu + expert-fb W2 + gate-combine into a per-tile
fp32 accumulator. Selection work on block 0 therefore overlaps the DMA stream
of blocks 1-3.

Other changes vs v4:
  - one-hot masks come from the host (index marshalling), removing the
    x-broadcast K=1 matmuls and the DVE compares;
  - softmax uses reciprocal_approx_fast (5x faster than DVE reciprocal, which
    stalled the PE ~1us per supertile) and gates are normalized BEFORE the
    per-expert broadcast, dropping the 128-row reciprocal broadcast and the
    final combine multiply;
  - gate-broadcast evac moved from ACT to DVE (ACT is near-saturated by the
    paired silu evacs in the phase loop).
"""

import os
import numpy as np
import ml_dtypes

import concourse.bass as bass
import concourse.mybir as mybir
import concourse.tile as tile
from concourse.bass_utils import run_bass_kernel_spmd

BF16 = ml_dtypes.bfloat16

B = 65536
V = 512
D = 1024
IN = 2048
E = 4
OUT = 128
NCORES = 8
F = E * D                 # 4096 features, expert-major (f = e*1024 + d)
KC = D // 128             # 8 contraction chunks per table
FB = 4                    # W1 feature blocks (1024 feats each == one expert)
NG = V // 128             # 4 vocab chunks per table

LAST_EXEC_NS = None       # set when BASSMOE_TRACE=1


def _legalize_waits(nc, max_waits=1):
    """This walrus build rejects instructions carrying more than ~1 sync-wait
    command; hoist all but the last wait onto single-wait NoOps."""
    for f in nc.m.functions:
        for bb in f.blocks:
            insts = bb.instructions
            if not any(
                inst.sync_info is not None and len(inst.sync_info.on_wait) > max_waits
                for inst in insts
            ):
                continue
            new = []
            for inst in insts:
                si = inst.sync_info
                waits = list(si.on_wait) if si is not None else []
                if len(waits) > max_waits:
                    for w in waits[:-max_waits]:
                        nop = mybir.InstNoOp(
                            name=f"legw-{nc.next_id()}", ins=[], outs=[]
                        )
                        nop.engine = inst.engine
                        nop.sync_info = mybir.SyncInfo(on_wait=[w], on_update=[])
                        new.append(nop)
                    inst.sync_info = mybir.SyncInfo(
                        on_wait=waits[-max_waits:], on_update=list(si.on_update)
                    )
                new.append(inst)
            bb.instructions = new


def build_program(visits, S, legalize=True):
    """visits: list of (offset, width, jk) with jk in {1, 2} naming which T1
    chunk table the supertile's i1 one-hots select from."""
    dt = mybir.dt
    f32, bf16 = dt.float32, dt.bfloat16
    AF = mybir.ActivationFunctionType
    ALU = mybir.AluOpType

    nc = bass.Bass()

    m0d = nc.dram_tensor("m0", [128, S], bf16, kind="ExternalInput")
    m1d = nc.dram_tensor("m1", [128, S], bf16, kind="ExternalInput")
    # embc[p, j, kc, v] = emb_tab(j)[chunk(j)*128 + v, kc*128 + p]
    embtd = nc.dram_tensor("embc", [128, 3, KC, 128], bf16, kind="ExternalInput")
    # w1m[t, fb, kc, p, ff] = W1flat[t*1024 + kc*128 + p, fb*1024 + ff]
    w1d = nc.dram_tensor("w1m", [2, FB, KC, 128, 1024], bf16, kind="ExternalInput")
    b1rd = nc.dram_tensor("b1row", [1, F], bf16, kind="ExternalInput")
    bgrd = nc.dram_tensor("bgrow", [1, E], bf16, kind="ExternalInput")
    wgd = nc.dram_tensor("wgm", [128, 2, KC, E], bf16, kind="ExternalInput")
    w2d = nc.dram_tensor("w2s", [128, E, KC, OUT], bf16, kind="ExternalInput")
    b2d = nc.dram_tensor("b2s", [128, E], f32, kind="ExternalInput")
    seld = nc.dram_tensor("sels", [E, E, 128], bf16, kind="ExternalInput")
    outd = nc.dram_tensor("out", [128, S], f32, kind="ExternalOutput")

    with tile.TileContext(nc) as tc:
        with (
            tc.tile_pool(name="const", bufs=1) as cpool,
            tc.tile_pool(name="w1st", bufs=2) as w1pool,
            tc.tile_pool(name="tt", bufs=2) as tpool,
            tc.tile_pool(name="hs", bufs=2) as hpool,
            tc.tile_pool(name="sm", bufs=2) as smpool,
            tc.tile_pool(name="gate", bufs=1) as gatepool,
            tc.tile_pool(name="accp", bufs=1) as apool,
            tc.tile_pool(name="tmpp", bufs=2) as tmpool,
            tc.tile_pool(name="gsc", bufs=2) as gspool,
            tc.tile_pool(name="pmm", bufs=2, space="PSUM") as pmm,
            tc.tile_pool(name="peo", bufs=1, space="PSUM") as peo,
            tc.tile_pool(name="pmisc", bufs=3, space="PSUM") as pmisc,
        ):
            # --- prologue DMAs, ordered by first use ---
            wg_sb = cpool.tile([128, 2, KC, E], bf16)
            nc.sync.dma_start(wg_sb[:], wgd[:])
            bgr_sb = cpool.tile([1, E], bf16)
            nc.sync.dma_start(bgr_sb[:], bgrd[:])
            embc_sb = cpool.tile([128, 3, KC, 128], bf16)
            nc.sync.dma_start(embc_sb[:], embtd[:])
            b1r_sb = cpool.tile([1, F], bf16)
            nc.sync.dma_start(b1r_sb[:], b1rd[:])
            w1t0 = w1pool.tile([128, KC, 1024], bf16, tag="w1")
            for kc in range(KC):
                nc.sync.dma_start(w1t0[:, kc, :], w1d[0, 0, kc])

            ones128_bf = cpool.tile([1, 128], bf16)
            nc.vector.memset(ones128_bf[:], 1.0)
            ones4_bf = cpool.tile([E, 1], bf16)
            nc.vector.memset(ones4_bf[:], 1.0)
            ones1x4 = cpool.tile([1, E], bf16)
            nc.vector.memset(ones1x4[:], 1.0)

            # --- exp'd gating chunk tables (all Exp before any Silu so the
            #     ACT table set loads exactly once each) ---
            g_sb = cpool.tile([128, 3, E], bf16)
            for j in range(3):
                tj = 0 if j == 0 else 1
                psg = pmisc.tile([128, E], f32, tag="misc")
                for kc in range(KC):
                    nc.tensor.matmul(
                        psg[:],
                        embc_sb[:, j, kc, :],
                        wg_sb[:, tj, kc, :],
                        start=(kc == 0),
                        stop=(kc == KC - 1 and j != 0),
                    )
                if j == 0:
                    # fold bg into chunk 0's table: psg += ones(v) x bg
                    nc.tensor.matmul(
                        psg[:], ones128_bf[:], bgr_sb[:], start=False, stop=True
                    )
                nc.scalar.activation(g_sb[:, j, :], psg[:], AF.Exp, bias=0.0)

            w1t1 = w1pool.tile([128, KC, 1024], bf16, tag="w1")
            for kc in range(KC):
                nc.sync.dma_start(w1t1[:, kc, :], w1d[1, 0, kc])

            # masks, piecewise in visit order so early supertiles start sooner
            m0_sb = cpool.tile([128, S], bf16)
            m1_sb = cpool.tile([128, S], bf16)
            for c0 in range(0, S, 1024):
                c1 = min(S, c0 + 1024)
                nc.sync.dma_start(m0_sb[:, c0:c1], m0d[:, c0:c1])
                nc.sync.dma_start(m1_sb[:, c0:c1], m1d[:, c0:c1])

            w2_sb = cpool.tile([128, E, KC, OUT], bf16)
            nc.sync.dma_start(w2_sb[:], w2d[:])
            b2_sb = cpool.tile([128, E], f32)
            nc.sync.dma_start(b2_sb[:], b2d[:])
            sel_sb = cpool.tile([E, E, 128], bf16)
            nc.sync.dma_start(sel_sb[:], seld[:])

            acc = {}
            gates = {}
            gparts = {}

            def emit_gating_part1(vi, off, w, jk):
                psa = pmisc.tile([E, w], f32, tag="misc")
                nc.tensor.matmul(
                    psa[:], g_sb[:, 0, :], m0_sb[:, off : off + w],
                    start=True, stop=True,
                )
                psb = pmisc.tile([E, w], f32, tag="misc")
                nc.tensor.matmul(
                    psb[:], g_sb[:, jk, :], m1_sb[:, off : off + w],
                    start=True, stop=True,
                )
                sa = smpool.tile([E, 512], f32, tag="s0")
                nc.vector.tensor_copy(sa[:, 0:w], psa[:])
                expt = smpool.tile([E, 512], bf16, tag="expt")
                nc.vector.tensor_tensor(expt[:, 0:w], sa[:, 0:w], psb[:], ALU.mult)
                gparts[vi] = expt

            def emit_gating_part2a(vi, off, w, jk):
                # sum-exp + fast reciprocal; the rb4 broadcast matmul is
                # deferred (part2b) so the PE never waits on the DVE chain
                expt = gparts[vi]
                sp = pmisc.tile([1, w], f32, tag="misc")
                nc.tensor.matmul(
                    sp[:], ones4_bf[:], expt[:, 0:w], start=True, stop=True
                )
                rec = smpool.tile([1, 512], f32, tag="rec")
                nc.vector.reciprocal_approx_fast(rec[:, 0:w], sp[:])
                recb = smpool.tile([1, 512], bf16, tag="recb")
                nc.vector.tensor_copy(recb[:, 0:w], rec[:, 0:w])
                gparts[vi] = (expt, recb)

            def emit_gating_part2b(vi, off, w, jk):
                expt, recb = gparts.pop(vi)
                rb4 = pmisc.tile([E, w], f32, tag="misc")
                nc.tensor.matmul(
                    rb4[:], ones1x4[:], recb[:, 0:w], start=True, stop=True
                )
                gt = gatepool.tile([E, 512], bf16, tag=f"gate{vi}")
                nc.vector.tensor_tensor(gt[:, 0:w], expt[:, 0:w], rb4[:], ALU.mult)
                gates[vi] = gt

            def emit_visit(fb, vi, off, w, jk, t_sb):
                if fb == 0:
                    emit_gating_part1(vi, off, w, jk)
                hs = []
                for pair in range(4):
                    hp = pmm.tile([128, 2, 512], f32, tag="mm")
                    for half in range(2):
                        lf = pair * 2 + half
                        fs = (lf % 4) * 128
                        nc.tensor.matmul(
                            hp[:, half, 0:w],
                            t_sb[:, 0, lf // 4, fs : fs + 128],
                            m0_sb[:, off : off + w],
                            start=True, stop=False,
                        )
                        nc.tensor.matmul(
                            hp[:, half, 0:w],
                            t_sb[:, jk, lf // 4, fs : fs + 128],
                            m1_sb[:, off : off + w],
                            start=False, stop=True,
                        )
                    hpair = hpool.tile([128, 2, 512], bf16, tag=f"h{pair}")
                    nc.scalar.activation(
                        hpair[:, :, 0:w], hp[:, :, 0:w], AF.Silu, bias=0.0
                    )
                    hs.append(hpair)
                    if fb == 0 and pair == 1:
                        emit_gating_part2a(vi, off, w, jk)
                if fb == 0:
                    emit_gating_part2b(vi, off, w, jk)
                eop = peo.tile([128, 512], f32, tag="eo")
                for dc in range(KC):
                    nc.tensor.matmul(
                        eop[:, 0:w],
                        w2_sb[:, fb, dc, :],
                        hs[dc // 2][:, dc % 2, 0:w],
                        start=(dc == 0),
                        stop=(dc == KC - 1),
                    )
                gp = pmisc.tile([128, w], f32, tag="misc")
                nc.tensor.matmul(
                    gp[:], sel_sb[:, fb, :], gates[vi][:, 0:w],
                    start=True, stop=True,
                )
                gs = gspool.tile([128, 512], bf16, tag="gs")
                nc.vector.tensor_copy(gs[:, 0:w], gp[:])
                if fb == 0:
                    a = apool.tile([128, 512], f32, tag=f"acc{vi}")
                    acc[vi] = a
                    nc.vector.scalar_tensor_tensor(
                        a[:, 0:w], eop[:, 0:w], b2_sb[:, fb : fb + 1],
                        gs[:, 0:w], ALU.add, ALU.mult,
                    )
                else:
                    tmp = tmpool.tile([128, 512], f32, tag="tmp")
                    nc.vector.scalar_tensor_tensor(
                        tmp[:, 0:w], eop[:, 0:w], b2_sb[:, fb : fb + 1],
                        gs[:, 0:w], ALU.add, ALU.mult,
                    )
                    nc.vector.tensor_add(acc[vi][:, 0:w], acc[vi][:, 0:w], tmp[:, 0:w])
                if fb == FB - 1:
                    nc.sync.dma_start(outd[:, off : off + w], acc[vi][:, 0:w])

            # --- expert-phase-major main loop ---
            for fb in range(FB):
                if fb == 0:
                    t0t, t1t = w1t0, w1t1
                else:
                    t0t = w1pool.tile([128, KC, 1024], bf16, tag="w1")
                    for kc in range(KC):
                        nc.sync.dma_start(t0t[:, kc, :], w1d[0, fb, kc])
                    t1t = w1pool.tile([128, KC, 1024], bf16, tag="w1")
                    for kc in range(KC):
                        nc.sync.dma_start(t1t[:, kc, :], w1d[1, fb, kc])
                # build the fb-slice of the 3 chunk tables
                t_sb = tpool.tile([128, 3, 2, 512], bf16, tag="t")
                for j in range(3):
                    w1t = t0t if j == 0 else t1t
                    ps = pmm.tile([128, 2, 512], f32, tag="mm")
                    for fp in range(2):
                        for kc in range(KC):
                            nc.tensor.matmul(
                                ps[:, fp, :],
                                embc_sb[:, j, kc, :],
                                w1t[:, kc, fp * 512 : (fp + 1) * 512],
                                start=(kc == 0),
                                stop=(kc == KC - 1 and j != 0),
                            )
                        if j == 0:
                            f0 = fb * 1024 + fp * 512
                            nc.tensor.matmul(
                                ps[:, fp, :], ones128_bf[:],
                                b1r_sb[:, f0 : f0 + 512],
                                start=False, stop=True,
                            )
                    nc.scalar.copy(t_sb[:, j, :, :], ps[:])
                for vi, (off, w, jk) in enumerate(visits):
                    emit_visit(fb, vi, off, w, jk, t_sb)

    if legalize:
        _legalize_waits(nc)
    mybir.codegen_inst_isa_subclasses(nc)
    return nc


def _roundup(n, m):
    return -(-n // m) * m


def assign_slots(x):
    """Global (i0//128, i1//128) bucketing: 16 classes -> 8 cores (2 each,
    sharing the i0 chunk). Returns per-core chunk ids, slot->token maps, and
    the shared visit layout."""
    x = np.asarray(x)
    c0 = x[:, 0] // 128
    c1 = x[:, 1] // 128
    cores = []
    for g in range(NG):
        idx = [np.nonzero((c0 == g) & (c1 == b))[0] for b in range(NG)]
        order = sorted(range(NG), key=lambda b: -len(idx[b]))
        for pa, pb in ((order[0], order[3]), (order[1], order[2])):
            if len(idx[pb]) > len(idx[pa]):
                pa, pb = pb, pa
            cores.append(
                dict(c0=g, c1a=pa, c1b=pb, ta=idx[pa], tb=idx[pb])
            )
    SA = _roundup(max(len(c["ta"]) for c in cores), 64)
    SB = _roundup(max(len(c["tb"]) for c in cores), 64)
    visits = []
    off = 0
    for span, jk in ((SA, 1), (SB, 2)):
        left = span
        while left > 0:
            w = min(512, left)
            visits.append((off, w, jk))
            off += w
            left -= w
    S = SA + SB
    slot_maps = []
    for c in cores:
        slots = np.full(S, -1, dtype=np.int64)
        slots[0 : len(c["ta"])] = c["ta"]
        slots[SA : SA + len(c["tb"])] = c["tb"]
        slot_maps.append(slots)
    return cores, slot_maps, visits, S


def marshal_inputs(x, emb0, emb1, W1, b1, W2, b2, Wg, bg, cores, slot_maps, S):
    x = np.asarray(x)
    emb0 = np.asarray(emb0)
    emb1 = np.asarray(emb1)

    shared = {}
    # W1flat[k, f] with f = e*1024 + d (expert-major features)
    w1flat = np.asarray(W1).transpose(1, 0, 2).reshape(IN, F)
    shared["w1m"] = np.ascontiguousarray(
        w1flat.reshape(2, KC, 128, FB, 1024).transpose(0, 3, 1, 2, 4).astype(BF16)
    )
    shared["b1row"] = np.ascontiguousarray(
        np.asarray(b1).reshape(1, F).astype(BF16)
    )
    shared["bgrow"] = np.ascontiguousarray(
        np.asarray(bg).reshape(1, E).astype(BF16)
    )
    shared["wgm"] = np.ascontiguousarray(
        np.asarray(Wg).reshape(2, KC, 128, E).transpose(2, 0, 1, 3).astype(BF16)
    )
    shared["w2s"] = np.ascontiguousarray(
        np.asarray(W2).reshape(E, KC, 128, OUT).transpose(2, 0, 1, 3).astype(BF16)
    )
    shared["b2s"] = np.ascontiguousarray(np.asarray(b2).T.astype(np.float32))
    shared["sels"] = np.ascontiguousarray(
        np.broadcast_to(np.eye(E, dtype=np.float32)[:, :, None], (E, E, 128)).astype(
            BF16
        )
    )

    in_maps = []
    for c, slots in zip(cores, slot_maps):
        # embc[p, j, kc, v] = emb_tab(j)[chunk(j)*128 + v, kc*128 + p]
        embc = np.empty((128, 3, KC, 128), dtype=BF16)
        for j, (tab, ch) in enumerate(
            ((emb0, c["c0"]), (emb1, c["c1a"]), (emb1, c["c1b"]))
        ):
            chunk = tab[ch * 128 : (ch + 1) * 128]  # [128v, 1024k]
            embc[:, j] = (
                chunk.reshape(128, KC, 128).transpose(2, 1, 0).astype(BF16)
            )
        # one-hot masks per slot (pad slots stay all-zero)
        m0 = np.zeros((128, S), dtype=BF16)
        m1 = np.zeros((128, S), dtype=BF16)
        pos = np.nonzero(slots >= 0)[0]
        tok = slots[pos]
        m0[x[tok, 0] % 128, pos] = 1.0
        m1[x[tok, 1] % 128, pos] = 1.0
        in_maps.append(
            {
                "m0": m0,
                "m1": m1,
                "embc": np.ascontiguousarray(embc),
                **shared,
            }
        )
    return in_maps


def kernel(x, emb0, emb1, W1, b1, W2, b2, Wg, bg):
    global LAST_EXEC_NS
    cores, slot_maps, visits, S = assign_slots(x)
    nc = build_program(visits, S)
    in_maps = marshal_inputs(
        x, emb0, emb1, W1, b1, W2, b2, Wg, bg, cores, slot_maps, S
    )
    trace = os.environ.get("BASSMOE_TRACE", "0") == "1"
    res = run_bass_kernel_spmd(nc, in_maps, list(range(NCORES)), trace=trace)
    LAST_EXEC_NS = res.exec_time_ns
    out = np.empty((B, OUT), dtype=np.float32)
    for c in range(NCORES):
        slots = slot_maps[c]
        pos = np.nonzero(slots >= 0)[0]
        r = res.results[c]["out"]  # [128, S]
        out[slots[pos], :] = r[:, pos].T
    return out


# revision 9
# speedup vs baseline: 1.2977x; 1.0250x over previous
"""MoE model via global vocab-pair bucketing + per-core chunk tables on 8 TRN2
cores.

v5 reworks v4's per-core bucketing into a GLOBAL (i0//128, i1//128) pair
bucketing: the host assigns each of the 16 chunk-pair classes to a core (2 per
core, sharing the i0 chunk), so each core precomputes only the 3 vocab-chunk
tables its tokens can touch (T0[c0], T1[c1a], T1[c1b]) instead of all 8 —
cutting the T = emb @ W1 precompute from 278k to ~104k PE cycles — and every
supertile is pure (2 selection matmuls per feature chunk, no mixed spill
tiles).

The main loop is EXPERT-PHASE-MAJOR: phase fb streams W1 block fb (2 x 2 MB),
builds the fb-slice of the 3 chunk tables, then for every supertile does the
8-fc selection + paired silu + expert-fb W2 + gate-combine into a per-tile
fp32 accumulator. Selection work on block 0 therefore overlaps the DMA stream
of blocks 1-3.

Other changes vs v4:
  - one-hot masks come from the host (index marshalling), removing the
    x-broadcast K=1 matmuls and the DVE compares;
  - softmax uses reciprocal_approx_fast (5x faster than DVE reciprocal, which
    stalled the PE ~1us per supertile) and gates are normalized BEFORE the
    per-expert broadcast, dropping the 128-row reciprocal broadcast and the
    final combine multiply;
  - gate-broadcast evac moved from ACT to DVE (ACT is near-saturated by the
    paired silu evacs in the phase loop).
"""

import os
import numpy as np
import ml_dtypes

import concourse.bass as bass
import concourse.mybir as mybir
import concourse.tile as tile
from concourse.bass_utils import run_bass_kernel_spmd

BF16 = ml_dtypes.bfloat16

B = 65536
V = 512
D = 1024
IN = 2048
E = 4
OUT = 128
NCORES = 8
F = E * D                 # 4096 features, expert-major (f = e*1024 + d)
KC = D // 128             # 8 contraction chunks per table
FB = 4                    # W1 feature blocks (1024 feats each == one expert)
NG = V // 128             # 4 vocab chunks per table

LAST_EXEC_NS = None       # set when BASSMOE_TRACE=1


def _legalize_waits(nc, max_waits=1):
    """This walrus build rejects instructions carrying more than ~1 sync-wait
    command; hoist all but the last wait onto single-wait NoOps."""
    for f in nc.m.functions:
        for bb in f.blocks:
            insts = bb.instructions
            if not any(
                inst.sync_info is not None and len(inst.sync_info.on_wait) > max_waits
                for inst in insts
            ):
                continue
            new = []
            for inst in insts:
                si = inst.sync_info
                waits = list(si.on_wait) if si is not None else []
                if len(waits) > max_waits:
                    for w in waits[:-max_waits]:
                        nop = mybir.InstNoOp(
                            name=f"legw-{nc.next_id()}", ins=[], outs=[]
                        )
                        nop.engine = inst.engine
                        nop.sync_info = mybir.SyncInfo(on_wait=[w], on_update=[])
                        new.append(nop)
                    inst.sync_info = mybir.SyncInfo(
                        on_wait=waits[-max_waits:], on_update=list(si.on_update)
                    )
                new.append(inst)
            bb.instructions = new


def build_program(visits, S, legalize=True):
    """visits: list of (offset, width, jk) with jk in {1, 2} naming which T1
    chunk table the supertile's i1 one-hots select from."""
    dt = mybir.dt
    f32, bf16 = dt.float32, dt.bfloat16
    AF = mybir.ActivationFunctionType
    ALU = mybir.AluOpType

    nc = bass.Bass()

    m0d = nc.dram_tensor("m0", [128, S], bf16, kind="ExternalInput")
    m1d = nc.dram_tensor("m1", [128, S], bf16, kind="ExternalInput")
    # embc[p, j, kc, v] = emb_tab(j)[chunk(j)*128 + v, kc*128 + p]
    embtd = nc.dram_tensor("embc", [128, 3, KC, 128], bf16, kind="ExternalInput")
    # w1m[t, fb, kc, p, ff] = W1flat[t*1024 + kc*128 + p, fb*1024 + ff]
    w1d = nc.dram_tensor("w1m", [2, FB, KC, 128, 1024], bf16, kind="ExternalInput")
    b1rd = nc.dram_tensor("b1row", [1, F], bf16, kind="ExternalInput")
    bgrd = nc.dram_tensor("bgrow", [1, E], bf16, kind="ExternalInput")
    wgd = nc.dram_tensor("wgm", [128, 2, KC, E], bf16, kind="ExternalInput")
    w2d = nc.dram_tensor("w2s", [128, E, KC, OUT], bf16, kind="ExternalInput")
    b2d = nc.dram_tensor("b2s", [128, E], f32, kind="ExternalInput")
    seld = nc.dram_tensor("sels", [128, E, 128], bf16, kind="ExternalInput")
    outd = nc.dram_tensor("out", [128, S], f32, kind="ExternalOutput")

    with tile.TileContext(nc) as tc:
        with (
            tc.tile_pool(name="const", bufs=1) as cpool,
            tc.tile_pool(name="w1st", bufs=2) as w1pool,
            tc.tile_pool(name="tt", bufs=2) as tpool,
            tc.tile_pool(name="hs", bufs=2) as hpool,
            tc.tile_pool(name="sm", bufs=2) as smpool,
            tc.tile_pool(name="gate", bufs=1) as gatepool,
            tc.tile_pool(name="accp", bufs=1) as apool,
            tc.tile_pool(name="tmpp", bufs=2) as tmpool,
            tc.tile_pool(name="gsc", bufs=2) as gspool,
            tc.tile_pool(name="pmm", bufs=2, space="PSUM") as pmm,
            tc.tile_pool(name="peo", bufs=1, space="PSUM") as peo,
            tc.tile_pool(name="pmisc", bufs=3, space="PSUM") as pmisc,
        ):
            # --- prologue DMAs, ordered by first use ---
            wg_sb = cpool.tile([128, 2, KC, E], bf16)
            nc.sync.dma_start(wg_sb[:], wgd[:])
            bgr_sb = cpool.tile([1, E], bf16)
            nc.sync.dma_start(bgr_sb[:], bgrd[:])
            embc_sb = cpool.tile([128, 3, KC, 128], bf16)
            for j in range(3):
                nc.sync.dma_start(embc_sb[:, j], embtd[:, j])
            b1r_sb = cpool.tile([1, F], bf16)
            nc.sync.dma_start(b1r_sb[:], b1rd[:])
            w1t0 = w1pool.tile([128, KC, 1024], bf16, tag="w1")
            for kc in range(KC):
                nc.sync.dma_start(w1t0[:, kc, :], w1d[0, 0, kc])

            ones128_bf = cpool.tile([1, 128], bf16)
            nc.vector.memset(ones128_bf[:], 1.0)
            # all-ones [128,128]: the sum-exp matmul then lands the sumexp
            # replicated on every output partition (broadcast for free)
            onessq = cpool.tile([128, 128], bf16)
            nc.vector.memset(onessq[:], 1.0)

            # --- exp'd gating chunk tables, padded to 128 stationary columns
            #     (zero cols 4..127) so every main-loop matmul keeps the
            #     (128,128) PE tile config — no quadrant-switch drains.
            #     All Exp before any Silu so the ACT table set loads once. ---
            g128_sb = cpool.tile([128, 3, 128], bf16)
            nc.vector.memset(g128_sb[:], 0.0)
            for j in range(3):
                tj = 0 if j == 0 else 1
                psg = pmisc.tile([128, E], f32, tag="misc")
                for kc in range(KC):
                    nc.tensor.matmul(
                        psg[:],
                        embc_sb[:, j, kc, :],
                        wg_sb[:, tj, kc, :],
                        start=(kc == 0),
                        stop=(kc == KC - 1 and j != 0),
                    )
                if j == 0:
                    # fold bg into chunk 0's table: psg += ones(v) x bg
                    nc.tensor.matmul(
                        psg[:], ones128_bf[:], bgr_sb[:], start=False, stop=True
                    )
                nc.scalar.activation(g128_sb[:, j, 0:E], psg[:], AF.Exp, bias=0.0)

            w1t1 = w1pool.tile([128, KC, 1024], bf16, tag="w1")
            for kc in range(KC):
                nc.sync.dma_start(w1t1[:, kc, :], w1d[1, 0, kc])

            # masks, piecewise in visit order so early supertiles start sooner
            m0_sb = cpool.tile([128, S], bf16)
            m1_sb = cpool.tile([128, S], bf16)
            for c0 in range(0, S, 1024):
                c1 = min(S, c0 + 1024)
                nc.sync.dma_start(m0_sb[:, c0:c1], m0d[:, c0:c1])
                nc.sync.dma_start(m1_sb[:, c0:c1], m1d[:, c0:c1])

            w2_sb = cpool.tile([128, E, KC, OUT], bf16)
            nc.sync.dma_start(w2_sb[:], w2d[:])
            b2_sb = cpool.tile([128, E], f32)
            nc.sync.dma_start(b2_sb[:], b2d[:])
            sel_sb = cpool.tile([128, E, 128], bf16)
            nc.sync.dma_start(sel_sb[:], seld[:])

            acc = {}
            gates = {}
            gparts = {}

            def emit_gating_part1(vi, off, w, jk):
                # exp-gate selections land on psum rows 0..3 (zero-padded
                # stationary cols keep rows 4..127 at exactly 0)
                psa = pmisc.tile([128, w], f32, tag="misc")
                nc.tensor.matmul(
                    psa[:], g128_sb[:, 0, :], m0_sb[:, off : off + w],
                    start=True, stop=True,
                )
                psb = pmisc.tile([128, w], f32, tag="misc")
                nc.tensor.matmul(
                    psb[:], g128_sb[:, jk, :], m1_sb[:, off : off + w],
                    start=True, stop=True,
                )
                sa = smpool.tile([128, 512], f32, tag="s0")
                nc.vector.tensor_copy(sa[:, 0:w], psa[:])
                expt = smpool.tile([128, 512], bf16, tag="expt")
                nc.vector.tensor_tensor(expt[:, 0:w], sa[:, 0:w], psb[:], ALU.mult)
                gparts[vi] = expt

            def emit_gating_part2a(vi, off, w, jk):
                # sum-exp via all-ones stationary: sumexp appears replicated
                # on every psum row, so 1/sumexp needs no broadcast matmul
                expt = gparts[vi]
                sp = pmisc.tile([128, w], f32, tag="misc")
                nc.tensor.matmul(
                    sp[:], onessq[:], expt[:, 0:w], start=True, stop=True
                )
                rec = smpool.tile([128, 512], f32, tag="rec")
                nc.vector.reciprocal_approx_fast(rec[:, 0:w], sp[:])
                gparts[vi] = (expt, rec)

            def emit_gating_part2b(vi, off, w, jk):
                expt, rec = gparts.pop(vi)
                gt = gatepool.tile([128, 512], bf16, tag=f"gate{vi}")
                nc.vector.tensor_tensor(gt[:, 0:w], expt[:, 0:w], rec[:, 0:w], ALU.mult)
                gates[vi] = gt

            def emit_visit(fb, vi, off, w, jk, t_sb):
                if fb == 0:
                    emit_gating_part1(vi, off, w, jk)
                hs = []
                for pair in range(4):
                    hp = pmm.tile([128, 2, 512], f32, tag="mm")
                    for half in range(2):
                        lf = pair * 2 + half
                        fs = (lf % 4) * 128
                        nc.tensor.matmul(
                            hp[:, half, 0:w],
                            t_sb[:, 0, lf // 4, fs : fs + 128],
                            m0_sb[:, off : off + w],
                            start=True, stop=False,
                        )
                        nc.tensor.matmul(
                            hp[:, half, 0:w],
                            t_sb[:, jk, lf // 4, fs : fs + 128],
                            m1_sb[:, off : off + w],
                            start=False, stop=True,
                        )
                    hpair = hpool.tile([128, 2, 512], bf16, tag=f"h{pair}")
                    nc.scalar.activation(
                        hpair[:, :, 0:w], hp[:, :, 0:w], AF.Silu, bias=0.0
                    )
                    hs.append(hpair)
                    if fb == 0 and pair == 1:
                        emit_gating_part2a(vi, off, w, jk)
                if fb == 0:
                    emit_gating_part2b(vi, off, w, jk)
                eop = peo.tile([128, 512], f32, tag="eo")
                for dc in range(KC):
                    nc.tensor.matmul(
                        eop[:, 0:w],
                        w2_sb[:, fb, dc, :],
                        hs[dc // 2][:, dc % 2, 0:w],
                        start=(dc == 0),
                        stop=(dc == KC - 1),
                    )
                gp = pmisc.tile([128, w], f32, tag="misc")
                nc.tensor.matmul(
                    gp[:], sel_sb[:, fb, :], gates[vi][:, 0:w],
                    start=True, stop=True,
                )
                gs = gspool.tile([128, 512], bf16, tag="gs")
                nc.vector.tensor_copy(gs[:, 0:w], gp[:])
                if fb == 0:
                    a = apool.tile([128, 512], f32, tag=f"acc{vi}")
                    acc[vi] = a
                    nc.vector.scalar_tensor_tensor(
                        a[:, 0:w], eop[:, 0:w], b2_sb[:, fb : fb + 1],
                        gs[:, 0:w], ALU.add, ALU.mult,
                    )
                else:
                    tmp = tmpool.tile([128, 512], f32, tag="tmp")
                    nc.vector.scalar_tensor_tensor(
                        tmp[:, 0:w], eop[:, 0:w], b2_sb[:, fb : fb + 1],
                        gs[:, 0:w], ALU.add, ALU.mult,
                    )
                    nc.vector.tensor_add(acc[vi][:, 0:w], acc[vi][:, 0:w], tmp[:, 0:w])
                if fb == FB - 1:
                    nc.sync.dma_start(outd[:, off : off + w], acc[vi][:, 0:w])

            # --- expert-phase-major main loop ---
            for fb in range(FB):
                if fb == 0:
                    t0t, t1t = w1t0, w1t1
                else:
                    t0t = w1pool.tile([128, KC, 1024], bf16, tag="w1")
                    for kc in range(KC):
                        nc.sync.dma_start(t0t[:, kc, :], w1d[0, fb, kc])
                    t1t = w1pool.tile([128, KC, 1024], bf16, tag="w1")
                    for kc in range(KC):
                        nc.sync.dma_start(t1t[:, kc, :], w1d[1, fb, kc])
                # build the fb-slice of the 3 chunk tables
                t_sb = tpool.tile([128, 3, 2, 512], bf16, tag="t")
                for j in range(3):
                    w1t = t0t if j == 0 else t1t
                    ps = pmm.tile([128, 2, 512], f32, tag="mm")
                    for fp in range(2):
                        for kc in range(KC):
                            nc.tensor.matmul(
                                ps[:, fp, :],
                                embc_sb[:, j, kc, :],
                                w1t[:, kc, fp * 512 : (fp + 1) * 512],
                                start=(kc == 0),
                                stop=(kc == KC - 1 and j != 0),
                            )
                        if j == 0:
                            f0 = fb * 1024 + fp * 512
                            nc.tensor.matmul(
                                ps[:, fp, :], ones128_bf[:],
                                b1r_sb[:, f0 : f0 + 512],
                                start=False, stop=True,
                            )
                    nc.scalar.copy(t_sb[:, j, :, :], ps[:])
                for vi, (off, w, jk) in enumerate(visits):
                    emit_visit(fb, vi, off, w, jk, t_sb)

    if legalize:
        _legalize_waits(nc)
    mybir.codegen_inst_isa_subclasses(nc)
    return nc


def _roundup(n, m):
    return -(-n // m) * m


def assign_slots(x):
    """Global (i0//128, i1//128) bucketing: 16 classes -> 8 cores (2 each,
    sharing the i0 chunk). Returns per-core chunk ids, slot->token maps, and
    the shared visit layout."""
    x = np.asarray(x)
    c0 = x[:, 0] // 128
    c1 = x[:, 1] // 128
    cores = []
    for g in range(NG):
        idx = [np.nonzero((c0 == g) & (c1 == b))[0] for b in range(NG)]
        order = sorted(range(NG), key=lambda b: -len(idx[b]))
        for pa, pb in ((order[0], order[3]), (order[1], order[2])):
            if len(idx[pb]) > len(idx[pa]):
                pa, pb = pb, pa
            cores.append(
                dict(c0=g, c1a=pa, c1b=pb, ta=idx[pa], tb=idx[pb])
            )
    SA = _roundup(max(len(c["ta"]) for c in cores), 64)
    SB = _roundup(max(len(c["tb"]) for c in cores), 64)
    visits = []
    off = 0
    for span, jk in ((SA, 1), (SB, 2)):
        left = span
        while left > 0:
            w = min(512, left)
            visits.append((off, w, jk))
            off += w
            left -= w
    S = SA + SB
    slot_maps = []
    for c in cores:
        slots = np.full(S, -1, dtype=np.int64)
        slots[0 : len(c["ta"])] = c["ta"]
        slots[SA : SA + len(c["tb"])] = c["tb"]
        slot_maps.append(slots)
    return cores, slot_maps, visits, S


def marshal_inputs(x, emb0, emb1, W1, b1, W2, b2, Wg, bg, cores, slot_maps, S):
    x = np.asarray(x)
    emb0 = np.asarray(emb0)
    emb1 = np.asarray(emb1)

    shared = {}
    # W1flat[k, f] with f = e*1024 + d (expert-major features)
    w1flat = np.asarray(W1).transpose(1, 0, 2).reshape(IN, F)
    shared["w1m"] = np.ascontiguousarray(
        w1flat.reshape(2, KC, 128, FB, 1024).transpose(0, 3, 1, 2, 4).astype(BF16)
    )
    shared["b1row"] = np.ascontiguousarray(
        np.asarray(b1).reshape(1, F).astype(BF16)
    )
    shared["bgrow"] = np.ascontiguousarray(
        np.asarray(bg).reshape(1, E).astype(BF16)
    )
    shared["wgm"] = np.ascontiguousarray(
        np.asarray(Wg).reshape(2, KC, 128, E).transpose(2, 0, 1, 3).astype(BF16)
    )
    shared["w2s"] = np.ascontiguousarray(
        np.asarray(W2).reshape(E, KC, 128, OUT).transpose(2, 0, 1, 3).astype(BF16)
    )
    shared["b2s"] = np.ascontiguousarray(np.asarray(b2).T.astype(np.float32))
    # sel128[p, e, o] = 1 iff p == e: a (128,128)-tile gate-row broadcast
    sel128 = np.zeros((128, E, 128), dtype=BF16)
    for e in range(E):
        sel128[e, e, :] = 1.0
    shared["sels"] = np.ascontiguousarray(sel128)

    in_maps = []
    for c, slots in zip(cores, slot_maps):
        # embc[p, j, kc, v] = emb_tab(j)[chunk(j)*128 + v, kc*128 + p]
        embc = np.empty((128, 3, KC, 128), dtype=BF16)
        for j, (tab, ch) in enumerate(
            ((emb0, c["c0"]), (emb1, c["c1a"]), (emb1, c["c1b"]))
        ):
            chunk = tab[ch * 128 : (ch + 1) * 128]  # [128v, 1024k]
            embc[:, j] = (
                chunk.reshape(128, KC, 128).transpose(2, 1, 0).astype(BF16)
            )
        # one-hot masks per slot (pad slots stay all-zero)
        m0 = np.zeros((128, S), dtype=BF16)
        m1 = np.zeros((128, S), dtype=BF16)
        pos = np.nonzero(slots >= 0)[0]
        tok = slots[pos]
        m0[x[tok, 0] % 128, pos] = 1.0
        m1[x[tok, 1] % 128, pos] = 1.0
        in_maps.append(
            {
                "m0": m0,
                "m1": m1,
                "embc": np.ascontiguousarray(embc),
                **shared,
            }
        )
    return in_maps


def kernel(x, emb0, emb1, W1, b1, W2, b2, Wg, bg):
    global LAST_EXEC_NS
    cores, slot_maps, visits, S = assign_slots(x)
    nc = build_program(visits, S)
    in_maps = marshal_inputs(
        x, emb0, emb1, W1, b1, W2, b2, Wg, bg, cores, slot_maps, S
    )
    trace = os.environ.get("BASSMOE_TRACE", "0") == "1"
    res = run_bass_kernel_spmd(nc, in_maps, list(range(NCORES)), trace=trace)
    LAST_EXEC_NS = res.exec_time_ns
    out = np.empty((B, OUT), dtype=np.float32)
    for c in range(NCORES):
        slots = slot_maps[c]
        pos = np.nonzero(slots >= 0)[0]
        r = res.results[c]["out"]  # [128, S]
        out[slots[pos], :] = r[:, pos].T
    return out


# revision 12
# speedup vs baseline: 1.3120x; 1.0110x over previous
"""MoE model via global vocab-pair bucketing + per-core chunk tables on 8 TRN2
cores.

v5 reworks v4's per-core bucketing into a GLOBAL (i0//128, i1//128) pair
bucketing: the host assigns each of the 16 chunk-pair classes to a core (2 per
core, sharing the i0 chunk), so each core precomputes only the 3 vocab-chunk
tables its tokens can touch (T0[c0], T1[c1a], T1[c1b]) instead of all 8 —
cutting the T = emb @ W1 precompute from 278k to ~104k PE cycles — and every
supertile is pure (2 selection matmuls per feature chunk, no mixed spill
tiles).

The main loop is EXPERT-PHASE-MAJOR: phase fb streams W1 block fb (2 x 2 MB),
builds the fb-slice of the 3 chunk tables, then for every supertile does the
8-fc selection + paired silu + expert-fb W2 + gate-combine into a per-tile
fp32 accumulator. Selection work on block 0 therefore overlaps the DMA stream
of blocks 1-3.

Other changes vs v4:
  - one-hot masks come from the host (index marshalling), removing the
    x-broadcast K=1 matmuls and the DVE compares;
  - softmax uses reciprocal_approx_fast (5x faster than DVE reciprocal, which
    stalled the PE ~1us per supertile) and gates are normalized BEFORE the
    per-expert broadcast, dropping the 128-row reciprocal broadcast and the
    final combine multiply;
  - gate-broadcast evac moved from ACT to DVE (ACT is near-saturated by the
    paired silu evacs in the phase loop).
"""

import os
import numpy as np
import ml_dtypes

import concourse.bass as bass
import concourse.mybir as mybir
import concourse.tile as tile
from concourse.bass_utils import run_bass_kernel_spmd

BF16 = ml_dtypes.bfloat16

B = 65536
V = 512
D = 1024
IN = 2048
E = 4
OUT = 128
NCORES = 8
F = E * D                 # 4096 features, expert-major (f = e*1024 + d)
KC = D // 128             # 8 contraction chunks per table
FB = 4                    # W1 feature blocks (1024 feats each == one expert)
NG = V // 128             # 4 vocab chunks per table

LAST_EXEC_NS = None       # set when BASSMOE_TRACE=1


def _legalize_waits(nc, max_waits=1):
    """This walrus build rejects instructions carrying more than ~1 sync-wait
    command; hoist all but the last wait onto single-wait NoOps."""
    for f in nc.m.functions:
        for bb in f.blocks:
            insts = bb.instructions
            if not any(
                inst.sync_info is not None and len(inst.sync_info.on_wait) > max_waits
                for inst in insts
            ):
                continue
            new = []
            for inst in insts:
                si = inst.sync_info
                waits = list(si.on_wait) if si is not None else []
                if len(waits) > max_waits:
                    for w in waits[:-max_waits]:
                        nop = mybir.InstNoOp(
                            name=f"legw-{nc.next_id()}", ins=[], outs=[]
                        )
                        nop.engine = inst.engine
                        nop.sync_info = mybir.SyncInfo(on_wait=[w], on_update=[])
                        new.append(nop)
                    inst.sync_info = mybir.SyncInfo(
                        on_wait=waits[-max_waits:], on_update=list(si.on_update)
                    )
                new.append(inst)
            bb.instructions = new


def build_program(visits, S, legalize=True):
    """visits: list of (offset, width, jk) with jk in {1, 2} naming which T1
    chunk table the supertile's i1 one-hots select from."""
    dt = mybir.dt
    f32, bf16 = dt.float32, dt.bfloat16
    AF = mybir.ActivationFunctionType
    ALU = mybir.AluOpType

    nc = bass.Bass()

    m0d = nc.dram_tensor("m0", [128, S], bf16, kind="ExternalInput")
    m1d = nc.dram_tensor("m1", [128, S], bf16, kind="ExternalInput")
    # embc[p, j, kc, v] = emb_tab(j)[chunk(j)*128 + v, kc*128 + p]
    embtd = nc.dram_tensor("embc", [128, 3, KC, 128], bf16, kind="ExternalInput")
    # w1m[t, fb, kc, p, ff] = W1flat[t*1024 + kc*128 + p, fb*1024 + ff]
    w1d = nc.dram_tensor("w1m", [2, FB, KC, 128, 1024], bf16, kind="ExternalInput")
    b1rd = nc.dram_tensor("b1row", [1, F], bf16, kind="ExternalInput")
    bgrd = nc.dram_tensor("bgrow", [1, E], bf16, kind="ExternalInput")
    wgd = nc.dram_tensor("wgm", [128, 2, KC, E], bf16, kind="ExternalInput")
    w2d = nc.dram_tensor("w2s", [128, E, KC, OUT], bf16, kind="ExternalInput")
    b2d = nc.dram_tensor("b2s", [128, E], f32, kind="ExternalInput")
    seld = nc.dram_tensor("sels", [128, E, 128], bf16, kind="ExternalInput")
    outd = nc.dram_tensor("out", [128, S], f32, kind="ExternalOutput")

    with tile.TileContext(nc) as tc:
        with (
            tc.tile_pool(name="const", bufs=1) as cpool,
            tc.tile_pool(name="w1st", bufs=2) as w1pool,
            tc.tile_pool(name="tt", bufs=2) as tpool,
            tc.tile_pool(name="hs", bufs=2) as hpool,
            tc.tile_pool(name="sm", bufs=2) as smpool,
            tc.tile_pool(name="gate", bufs=1) as gatepool,
            tc.tile_pool(name="accp", bufs=1) as apool,
            tc.tile_pool(name="tmpp", bufs=2) as tmpool,
            tc.tile_pool(name="gsc", bufs=2) as gspool,
            tc.tile_pool(name="pmm", bufs=2, space="PSUM") as pmm,
            tc.tile_pool(name="peo", bufs=1, space="PSUM") as peo,
            tc.tile_pool(name="pmisc", bufs=3, space="PSUM") as pmisc,
        ):
            # --- prologue DMAs, ordered by first use ---
            wg_sb = cpool.tile([128, 2, KC, E], bf16)
            nc.sync.dma_start(wg_sb[:], wgd[:])
            bgr_sb = cpool.tile([1, E], bf16)
            nc.sync.dma_start(bgr_sb[:], bgrd[:])
            embc_sb = cpool.tile([128, 3, KC, 128], bf16)
            for kc in range(KC):
                nc.sync.dma_start(embc_sb[:, 0, kc], embtd[:, 0, kc])
            for j in (1, 2):
                nc.sync.dma_start(embc_sb[:, j], embtd[:, j])
            b1r_sb = cpool.tile([1, F], bf16)
            nc.sync.dma_start(b1r_sb[:], b1rd[:])
            w1t0 = w1pool.tile([128, KC, 1024], bf16, tag="w1")
            for kc in range(KC):
                nc.sync.dma_start(w1t0[:, kc, :], w1d[0, 0, kc])

            ones128_bf = cpool.tile([1, 128], bf16)
            nc.vector.memset(ones128_bf[:], 1.0)
            # all-ones [128,128]: the sum-exp matmul then lands the sumexp
            # replicated on every output partition (broadcast for free)
            onessq = cpool.tile([128, 128], bf16)
            nc.vector.memset(onessq[:], 1.0)

            # --- exp'd gating chunk tables, padded to 128 stationary columns
            #     (zero cols 4..127) so every main-loop matmul keeps the
            #     (128,128) PE tile config — no quadrant-switch drains.
            #     All Exp before any Silu so the ACT table set loads once. ---
            g128_sb = cpool.tile([128, 3, 128], bf16)
            nc.vector.memset(g128_sb[:], 0.0)
            for j in range(3):
                tj = 0 if j == 0 else 1
                psg = pmisc.tile([128, E], f32, tag="misc")
                for kc in range(KC):
                    nc.tensor.matmul(
                        psg[:],
                        embc_sb[:, j, kc, :],
                        wg_sb[:, tj, kc, :],
                        start=(kc == 0),
                        stop=(kc == KC - 1 and j != 0),
                    )
                if j == 0:
                    # fold bg into chunk 0's table: psg += ones(v) x bg
                    nc.tensor.matmul(
                        psg[:], ones128_bf[:], bgr_sb[:], start=False, stop=True
                    )
                nc.scalar.activation(g128_sb[:, j, 0:E], psg[:], AF.Exp, bias=0.0)

            w1t1 = w1pool.tile([128, KC, 1024], bf16, tag="w1")
            for kc in range(KC):
                nc.sync.dma_start(w1t1[:, kc, :], w1d[1, 0, kc])

            # masks, piecewise in visit order so early supertiles start sooner
            m0_sb = cpool.tile([128, S], bf16)
            m1_sb = cpool.tile([128, S], bf16)
            for c0 in range(0, S, 1024):
                c1 = min(S, c0 + 1024)
                nc.sync.dma_start(m0_sb[:, c0:c1], m0d[:, c0:c1])
                nc.sync.dma_start(m1_sb[:, c0:c1], m1d[:, c0:c1])

            w2_sb = cpool.tile([128, E, KC, OUT], bf16)
            nc.sync.dma_start(w2_sb[:], w2d[:])
            b2_sb = cpool.tile([128, E], f32)
            nc.sync.dma_start(b2_sb[:], b2d[:])
            sel_sb = cpool.tile([128, E, 128], bf16)
            nc.sync.dma_start(sel_sb[:], seld[:])

            acc = {}
            gates = {}
            gparts = {}

            def emit_gating_part1(vi, off, w, jk):
                # exp-gate selections land on psum rows 0..3 (zero-padded
                # stationary cols keep rows 4..127 at exactly 0)
                psa = pmisc.tile([128, w], f32, tag="misc")
                nc.tensor.matmul(
                    psa[:], g128_sb[:, 0, :], m0_sb[:, off : off + w],
                    start=True, stop=True,
                )
                psb = pmisc.tile([128, w], f32, tag="misc")
                nc.tensor.matmul(
                    psb[:], g128_sb[:, jk, :], m1_sb[:, off : off + w],
                    start=True, stop=True,
                )
                sa = smpool.tile([128, 512], f32, tag="s0")
                nc.vector.tensor_copy(sa[:, 0:w], psa[:])
                expt = smpool.tile([128, 512], bf16, tag="expt")
                nc.vector.tensor_tensor(expt[:, 0:w], sa[:, 0:w], psb[:], ALU.mult)
                gparts[vi] = expt

            def emit_gating_part2a(vi, off, w, jk):
                # sum-exp via all-ones stationary: sumexp appears replicated
                # on every psum row, so 1/sumexp needs no broadcast matmul
                expt = gparts[vi]
                sp = pmisc.tile([128, w], f32, tag="misc")
                nc.tensor.matmul(
                    sp[:], onessq[:], expt[:, 0:w], start=True, stop=True
                )
                rec = smpool.tile([128, 512], f32, tag="rec")
                nc.vector.reciprocal_approx_fast(rec[:, 0:w], sp[:])
                gparts[vi] = (expt, rec)

            def emit_gating_part2b(vi, off, w, jk):
                expt, rec = gparts.pop(vi)
                gt = gatepool.tile([128, 512], bf16, tag=f"gate{vi}")
                nc.vector.tensor_tensor(gt[:, 0:w], expt[:, 0:w], rec[:, 0:w], ALU.mult)
                gates[vi] = gt

            def emit_w2(eop, fb, w, hpair, p):
                for half in range(2):
                    dc = p * 2 + half
                    nc.tensor.matmul(
                        eop[:, 0:w],
                        w2_sb[:, fb, dc, :],
                        hpair[:, half, 0:w],
                        start=(dc == 0),
                        stop=(dc == KC - 1),
                    )

            def emit_visit(fb, vi, off, w, jk, t_sb):
                if fb == 0:
                    emit_gating_part1(vi, off, w, jk)
                # W2 matmuls for h[p-1] are interleaved after selection pair p:
                # this widens the pmm rotation window past the ~1.1us silu
                # evac so the PE never waits on the ACT engine
                hs = []
                eop = peo.tile([128, 512], f32, tag="eo")
                for pair in range(4):
                    hp = pmm.tile([128, 2, 512], f32, tag="mm")
                    for half in range(2):
                        lf = pair * 2 + half
                        fs = (lf % 4) * 128
                        nc.tensor.matmul(
                            hp[:, half, 0:w],
                            t_sb[:, 0, lf // 4, fs : fs + 128],
                            m0_sb[:, off : off + w],
                            start=True, stop=False,
                        )
                        nc.tensor.matmul(
                            hp[:, half, 0:w],
                            t_sb[:, jk, lf // 4, fs : fs + 128],
                            m1_sb[:, off : off + w],
                            start=False, stop=True,
                        )
                    hpair = hpool.tile([128, 2, 512], bf16, tag=f"h{pair}")
                    nc.scalar.activation(
                        hpair[:, :, 0:w], hp[:, :, 0:w], AF.Silu, bias=0.0
                    )
                    hs.append(hpair)
                    if fb == 0 and pair == 1:
                        emit_gating_part2a(vi, off, w, jk)
                    if pair >= 1:
                        emit_w2(eop, fb, w, hs[pair - 1], pair - 1)
                if fb == 0:
                    emit_gating_part2b(vi, off, w, jk)
                emit_w2(eop, fb, w, hs[3], 3)
                gp = pmisc.tile([128, w], f32, tag="misc")
                nc.tensor.matmul(
                    gp[:], sel_sb[:, fb, :], gates[vi][:, 0:w],
                    start=True, stop=True,
                )
                gs = gspool.tile([128, 512], bf16, tag="gs")
                nc.vector.tensor_copy(gs[:, 0:w], gp[:])
                if fb == 0:
                    a = apool.tile([128, 512], f32, tag=f"acc{vi}")
                    acc[vi] = a
                    nc.vector.scalar_tensor_tensor(
                        a[:, 0:w], eop[:, 0:w], b2_sb[:, fb : fb + 1],
                        gs[:, 0:w], ALU.add, ALU.mult,
                    )
                else:
                    tmp = tmpool.tile([128, 512], f32, tag="tmp")
                    nc.vector.scalar_tensor_tensor(
                        tmp[:, 0:w], eop[:, 0:w], b2_sb[:, fb : fb + 1],
                        gs[:, 0:w], ALU.add, ALU.mult,
                    )
                    nc.vector.tensor_add(acc[vi][:, 0:w], acc[vi][:, 0:w], tmp[:, 0:w])
                if fb == FB - 1:
                    nc.sync.dma_start(outd[:, off : off + w], acc[vi][:, 0:w])

            # --- expert-phase-major main loop ---
            for fb in range(FB):
                if fb == 0:
                    t0t, t1t = w1t0, w1t1
                else:
                    t0t = w1pool.tile([128, KC, 1024], bf16, tag="w1")
                    for kc in range(KC):
                        nc.sync.dma_start(t0t[:, kc, :], w1d[0, fb, kc])
                    t1t = w1pool.tile([128, KC, 1024], bf16, tag="w1")
                    for kc in range(KC):
                        nc.sync.dma_start(t1t[:, kc, :], w1d[1, fb, kc])
                # build the fb-slice of the 3 chunk tables
                t_sb = tpool.tile([128, 3, 2, 512], bf16, tag="t")
                for j in range(3):
                    w1t = t0t if j == 0 else t1t
                    ps = pmm.tile([128, 2, 512], f32, tag="mm")
                    for fp in range(2):
                        for kc in range(KC):
                            nc.tensor.matmul(
                                ps[:, fp, :],
                                embc_sb[:, j, kc, :],
                                w1t[:, kc, fp * 512 : (fp + 1) * 512],
                                start=(kc == 0),
                                stop=(kc == KC - 1 and j != 0),
                            )
                        if j == 0:
                            f0 = fb * 1024 + fp * 512
                            nc.tensor.matmul(
                                ps[:, fp, :], ones128_bf[:],
                                b1r_sb[:, f0 : f0 + 512],
                                start=False, stop=True,
                            )
                    nc.scalar.copy(t_sb[:, j, :, :], ps[:])
                for vi, (off, w, jk) in enumerate(visits):
                    emit_visit(fb, vi, off, w, jk, t_sb)

    if legalize:
        _legalize_waits(nc)
    mybir.codegen_inst_isa_subclasses(nc)
    return nc


def _roundup(n, m):
    return -(-n // m) * m


def assign_slots(x):
    """Global (i0//128, i1//128) bucketing: 16 classes -> 8 cores (2 each,
    sharing the i0 chunk). Returns per-core chunk ids, slot->token maps, and
    the shared visit layout."""
    x = np.asarray(x)
    c0 = x[:, 0] // 128
    c1 = x[:, 1] // 128
    cores = []
    for g in range(NG):
        idx = [np.nonzero((c0 == g) & (c1 == b))[0] for b in range(NG)]
        order = sorted(range(NG), key=lambda b: -len(idx[b]))
        for pa, pb in ((order[0], order[3]), (order[1], order[2])):
            if len(idx[pb]) > len(idx[pa]):
                pa, pb = pb, pa
            cores.append(
                dict(c0=g, c1a=pa, c1b=pb, ta=idx[pa], tb=idx[pb])
            )
    SA = _roundup(max(len(c["ta"]) for c in cores), 64)
    SB = _roundup(max(len(c["tb"]) for c in cores), 64)
    visits = []
    off = 0
    for span, jk in ((SA, 1), (SB, 2)):
        left = span
        while left > 0:
            w = min(512, left)
            visits.append((off, w, jk))
            off += w
            left -= w
    S = SA + SB
    slot_maps = []
    for c in cores:
        slots = np.full(S, -1, dtype=np.int64)
        slots[0 : len(c["ta"])] = c["ta"]
        slots[SA : SA + len(c["tb"])] = c["tb"]
        slot_maps.append(slots)
    return cores, slot_maps, visits, S


def marshal_inputs(x, emb0, emb1, W1, b1, W2, b2, Wg, bg, cores, slot_maps, S):
    x = np.asarray(x)
    emb0 = np.asarray(emb0)
    emb1 = np.asarray(emb1)

    shared = {}
    # W1flat[k, f] with f = e*1024 + d (expert-major features)
    w1flat = np.asarray(W1).transpose(1, 0, 2).reshape(IN, F)
    shared["w1m"] = np.ascontiguousarray(
        w1flat.reshape(2, KC, 128, FB, 1024).transpose(0, 3, 1, 2, 4).astype(BF16)
    )
    shared["b1row"] = np.ascontiguousarray(
        np.asarray(b1).reshape(1, F).astype(BF16)
    )
    shared["bgrow"] = np.ascontiguousarray(
        np.asarray(bg).reshape(1, E).astype(BF16)
    )
    shared["wgm"] = np.ascontiguousarray(
        np.asarray(Wg).reshape(2, KC, 128, E).transpose(2, 0, 1, 3).astype(BF16)
    )
    shared["w2s"] = np.ascontiguousarray(
        np.asarray(W2).reshape(E, KC, 128, OUT).transpose(2, 0, 1, 3).astype(BF16)
    )
    shared["b2s"] = np.ascontiguousarray(np.asarray(b2).T.astype(np.float32))
    # sel128[p, e, o] = 1 iff p == e: a (128,128)-tile gate-row broadcast
    sel128 = np.zeros((128, E, 128), dtype=BF16)
    for e in range(E):
        sel128[e, e, :] = 1.0
    shared["sels"] = np.ascontiguousarray(sel128)

    in_maps = []
    for c, slots in zip(cores, slot_maps):
        # embc[p, j, kc, v] = emb_tab(j)[chunk(j)*128 + v, kc*128 + p]
        embc = np.empty((128, 3, KC, 128), dtype=BF16)
        for j, (tab, ch) in enumerate(
            ((emb0, c["c0"]), (emb1, c["c1a"]), (emb1, c["c1b"]))
        ):
            chunk = tab[ch * 128 : (ch + 1) * 128]  # [128v, 1024k]
            embc[:, j] = (
                chunk.reshape(128, KC, 128).transpose(2, 1, 0).astype(BF16)
            )
        # one-hot masks per slot (pad slots stay all-zero)
        m0 = np.zeros((128, S), dtype=BF16)
        m1 = np.zeros((128, S), dtype=BF16)
        pos = np.nonzero(slots >= 0)[0]
        tok = slots[pos]
        m0[x[tok, 0] % 128, pos] = 1.0
        m1[x[tok, 1] % 128, pos] = 1.0
        in_maps.append(
            {
                "m0": m0,
                "m1": m1,
                "embc": np.ascontiguousarray(embc),
                **shared,
            }
        )
    return in_maps


def kernel(x, emb0, emb1, W1, b1, W2, b2, Wg, bg):
    global LAST_EXEC_NS
    cores, slot_maps, visits, S = assign_slots(x)
    nc = build_program(visits, S)
    in_maps = marshal_inputs(
        x, emb0, emb1, W1, b1, W2, b2, Wg, bg, cores, slot_maps, S
    )
    trace = os.environ.get("BASSMOE_TRACE", "0") == "1"
    res = run_bass_kernel_spmd(nc, in_maps, list(range(NCORES)), trace=trace)
    LAST_EXEC_NS = res.exec_time_ns
    out = np.empty((B, OUT), dtype=np.float32)
    for c in range(NCORES):
        slots = slot_maps[c]
        pos = np.nonzero(slots >= 0)[0]
        r = res.results[c]["out"]  # [128, S]
        out[slots[pos], :] = r[:, pos].T
    return out


# revision 19
# speedup vs baseline: 1.4235x; 1.0850x over previous
"""MoE model via global vocab-pair bucketing + per-core chunk tables on 8 TRN2
cores.

v5 reworks v4's per-core bucketing into a GLOBAL (i0//128, i1//128) pair
bucketing: the host assigns each of the 16 chunk-pair classes to a core (2 per
core, sharing the i0 chunk), so each core precomputes only the 3 vocab-chunk
tables its tokens can touch (T0[c0], T1[c1a], T1[c1b]) instead of all 8 —
cutting the T = emb @ W1 precompute from 278k to ~104k PE cycles — and every
supertile is pure (2 selection matmuls per feature chunk, no mixed spill
tiles).

The main loop is EXPERT-PHASE-MAJOR: phase fb streams W1 block fb (2 x 2 MB),
builds the fb-slice of the 3 chunk tables, then for every supertile does the
8-fc selection + paired silu + expert-fb W2 + gate-combine into a per-tile
fp32 accumulator. Selection work on block 0 therefore overlaps the DMA stream
of blocks 1-3.

Other changes vs v4:
  - one-hot masks come from the host (index marshalling), removing the
    x-broadcast K=1 matmuls and the DVE compares;
  - softmax uses reciprocal_approx_fast (5x faster than DVE reciprocal, which
    stalled the PE ~1us per supertile) and gates are normalized BEFORE the
    per-expert broadcast, dropping the 128-row reciprocal broadcast and the
    final combine multiply;
  - gate-broadcast evac moved from ACT to DVE (ACT is near-saturated by the
    paired silu evacs in the phase loop).
"""

import os
import numpy as np
import ml_dtypes

import concourse.bass as bass
import concourse.mybir as mybir
import concourse.tile as tile
from concourse.bass_utils import run_bass_kernel_spmd

BF16 = ml_dtypes.bfloat16

B = 65536
V = 512
D = 1024
IN = 2048
E = 4
OUT = 128
NCORES = 8
F = E * D                 # 4096 features, expert-major (f = e*1024 + d)
KC = D // 128             # 8 contraction chunks per table
FB = 4                    # W1 feature blocks (1024 feats each == one expert)
NG = V // 128             # 4 vocab chunks per table

LAST_EXEC_NS = None       # set when BASSMOE_TRACE=1


def _legalize_waits(nc, max_waits=1):
    """This walrus build rejects instructions carrying more than ~1 sync-wait
    command; hoist all but the last wait onto single-wait NoOps."""
    for f in nc.m.functions:
        for bb in f.blocks:
            insts = bb.instructions
            if not any(
                inst.sync_info is not None and len(inst.sync_info.on_wait) > max_waits
                for inst in insts
            ):
                continue
            new = []
            for inst in insts:
                si = inst.sync_info
                waits = list(si.on_wait) if si is not None else []
                if len(waits) > max_waits:
                    for w in waits[:-max_waits]:
                        nop = mybir.InstNoOp(
                            name=f"legw-{nc.next_id()}", ins=[], outs=[]
                        )
                        nop.engine = inst.engine
                        nop.sync_info = mybir.SyncInfo(on_wait=[w], on_update=[])
                        new.append(nop)
                    inst.sync_info = mybir.SyncInfo(
                        on_wait=waits[-max_waits:], on_update=list(si.on_update)
                    )
                new.append(inst)
            bb.instructions = new


def build_program(visits, S, legalize=True):
    """visits: list of (offset, width, jk) with jk in {1, 2} naming which T1
    chunk table the supertile's i1 one-hots select from."""
    dt = mybir.dt
    f32, bf16 = dt.float32, dt.bfloat16
    AF = mybir.ActivationFunctionType
    ALU = mybir.AluOpType

    nc = bass.Bass()

    m0d = nc.dram_tensor("m0", [128, S], bf16, kind="ExternalInput")
    m1d = nc.dram_tensor("m1", [128, S], bf16, kind="ExternalInput")
    # embc[p, j, kc, v] = emb_tab(j)[chunk(j)*128 + v, kc*128 + p]
    embtd = nc.dram_tensor("embc", [128, 3, KC, 128], bf16, kind="ExternalInput")
    # w1m[t, fb, kc, p, ff] = W1flat[t*1024 + kc*128 + p, fb*1024 + ff]
    w1d = nc.dram_tensor("w1m", [2, FB, KC, 128, 1024], bf16, kind="ExternalInput")
    b1rd = nc.dram_tensor("b1row", [1, F], bf16, kind="ExternalInput")
    bgrd = nc.dram_tensor("bgrow", [1, E], bf16, kind="ExternalInput")
    wgd = nc.dram_tensor("wgm", [128, 2, KC, E], bf16, kind="ExternalInput")
    w2d = nc.dram_tensor("w2s", [128, E, KC, OUT], bf16, kind="ExternalInput")
    b2d = nc.dram_tensor("b2s", [128, E], f32, kind="ExternalInput")
    seld = nc.dram_tensor("sels", [128, E, 128], bf16, kind="ExternalInput")
    outd = nc.dram_tensor("out", [128, S], f32, kind="ExternalOutput")

    with tile.TileContext(nc) as tc:
        with (
            tc.tile_pool(name="const", bufs=1) as cpool,
            tc.tile_pool(name="w1st", bufs=2) as w1pool,
            tc.tile_pool(name="tt", bufs=2) as tpool,
            tc.tile_pool(name="hs", bufs=2) as hpool,
            tc.tile_pool(name="sm", bufs=2) as smpool,
            tc.tile_pool(name="gate", bufs=1) as gatepool,
            tc.tile_pool(name="accp", bufs=1) as apool,
            tc.tile_pool(name="tmpp", bufs=2) as tmpool,
            tc.tile_pool(name="gsc", bufs=2) as gspool,
            tc.tile_pool(name="pmm", bufs=2, space="PSUM") as pmm,
            tc.tile_pool(name="peo", bufs=1, space="PSUM") as peo,
            tc.tile_pool(name="pmisc", bufs=3, space="PSUM") as pmisc,
        ):
            # --- prologue DMAs, ordered by first use ---
            wg_sb = cpool.tile([128, 2, KC, E], bf16)
            nc.sync.dma_start(wg_sb[:], wgd[:])
            bgr_sb = cpool.tile([1, E], bf16)
            nc.sync.dma_start(bgr_sb[:], bgrd[:])
            embc_sb = cpool.tile([128, 3, KC, 128], bf16)
            for kc in range(KC):
                nc.sync.dma_start(embc_sb[:, 0, kc], embtd[:, 0, kc])
            w1t0 = w1pool.tile([128, KC, 1024], bf16, tag="w1")
            for kc in range(KC):
                nc.sync.dma_start(w1t0[:, kc, :], w1d[0, 0, kc])
            b1r_sb = cpool.tile([1, F], bf16)
            nc.sync.dma_start(b1r_sb[:], b1rd[:])

            ones128_bf = cpool.tile([1, 128], bf16)
            nc.vector.memset(ones128_bf[:], 1.0)
            # all-ones [128,128]: the sum-exp matmul then lands the sumexp
            # replicated on every output partition (broadcast for free)
            onessq = cpool.tile([128, 128], bf16)
            nc.vector.memset(onessq[:], 1.0)
            # exp'd gating chunk tables, padded to 128 stationary columns
            # (zero cols 4..127) so every main-loop matmul keeps the
            # (128,128) PE tile config — no quadrant-switch drains
            g128_sb = cpool.tile([128, 3, 128], bf16)
            nc.vector.memset(g128_sb[:], 0.0)

            # w1t1 + the remaining embc chunks stream per-kc interleaved so
            # the fb0/j1 precompute can chase the DMA
            w1t1 = w1pool.tile([128, KC, 1024], bf16, tag="w1")
            for kc in range(KC):
                nc.sync.dma_start(w1t1[:, kc, :], w1d[1, 0, kc])
                nc.sync.dma_start(embc_sb[:, 1, kc], embtd[:, 1, kc])
                nc.sync.dma_start(embc_sb[:, 2, kc], embtd[:, 2, kc])

            def emit_g(j):
                # one Exp table chunk; all Exp run before any Silu so the ACT
                # table set loads exactly once each
                tj = 0 if j == 0 else 1
                psg = pmisc.tile([128, E], f32, tag="misc")
                for kc in range(KC):
                    nc.tensor.matmul(
                        psg[:],
                        embc_sb[:, j, kc, :],
                        wg_sb[:, tj, kc, :],
                        start=(kc == 0),
                        stop=(kc == KC - 1 and j != 0),
                    )
                if j == 0:
                    # fold bg into chunk 0's table: psg += ones(v) x bg
                    nc.tensor.matmul(
                        psg[:], ones128_bf[:], bgr_sb[:], start=False, stop=True
                    )
                nc.scalar.activation(g128_sb[:, j, 0:E], psg[:], AF.Exp, bias=0.0)

            # masks, piecewise in visit order so early supertiles start sooner
            m0_sb = cpool.tile([128, S], bf16)
            m1_sb = cpool.tile([128, S], bf16)
            for c0 in range(0, S, 1024):
                c1 = min(S, c0 + 1024)
                nc.sync.dma_start(m0_sb[:, c0:c1], m0d[:, c0:c1])
                nc.sync.dma_start(m1_sb[:, c0:c1], m1d[:, c0:c1])

            w2_sb = cpool.tile([128, E, KC, OUT], bf16)
            nc.sync.dma_start(w2_sb[:], w2d[:])
            b2_sb = cpool.tile([128, E], f32)
            nc.sync.dma_start(b2_sb[:], b2d[:])
            sel_sb = cpool.tile([128, E, 128], bf16)
            nc.sync.dma_start(sel_sb[:], seld[:])

            acc = {}
            gates = {}
            gparts = {}

            def emit_gating_part1(vi, off, w, jk):
                # exp-gate selections land on psum rows 0..3 (zero-padded
                # stationary cols keep rows 4..127 at exactly 0)
                psa = pmisc.tile([128, w], f32, tag="misc")
                nc.tensor.matmul(
                    psa[:], g128_sb[:, 0, :], m0_sb[:, off : off + w],
                    start=True, stop=True,
                )
                psb = pmisc.tile([128, w], f32, tag="misc")
                nc.tensor.matmul(
                    psb[:], g128_sb[:, jk, :], m1_sb[:, off : off + w],
                    start=True, stop=True,
                )
                sa = smpool.tile([128, 512], f32, tag="s0")
                nc.vector.tensor_copy(sa[:, 0:w], psa[:])
                expt = smpool.tile([128, 512], bf16, tag="expt")
                nc.vector.tensor_tensor(expt[:, 0:w], sa[:, 0:w], psb[:], ALU.mult)
                gparts[vi] = expt

            def emit_gating_part2a(vi, off, w, jk):
                # sum-exp via all-ones stationary: sumexp appears replicated
                # on every psum row, so 1/sumexp needs no broadcast matmul
                expt = gparts[vi]
                sp = pmisc.tile([128, w], f32, tag="misc")
                nc.tensor.matmul(
                    sp[:], onessq[:], expt[:, 0:w], start=True, stop=True
                )
                rec = smpool.tile([128, 512], f32, tag="rec")
                nc.vector.reciprocal_approx_fast(rec[:, 0:w], sp[:])
                gparts[vi] = (expt, rec)

            def emit_gating_part2b(vi, off, w, jk):
                expt, rec = gparts.pop(vi)
                gt = gatepool.tile([128, 512], bf16, tag=f"gate{vi}")
                nc.vector.tensor_tensor(gt[:, 0:w], expt[:, 0:w], rec[:, 0:w], ALU.mult)
                gates[vi] = gt

            def emit_w2(eop, fb, w, hpair, p):
                for half in range(2):
                    dc = p * 2 + half
                    nc.tensor.matmul(
                        eop[:, 0:w],
                        w2_sb[:, fb, dc, :],
                        hpair[:, half, 0:w],
                        start=(dc == 0),
                        stop=(dc == KC - 1),
                    )

            def emit_visit(fb, vi, off, w, jk, t_sb):
                if fb == 0:
                    emit_gating_part1(vi, off, w, jk)
                # W2 matmuls for h[p-1] are interleaved after selection pair p:
                # this widens the pmm rotation window past the ~1.1us silu
                # evac so the PE never waits on the ACT engine
                hs = []
                eop = peo.tile([128, 512], f32, tag="eo")
                for pair in range(4):
                    hp = pmm.tile([128, 2, 512], f32, tag="mm")
                    for half in range(2):
                        lf = pair * 2 + half
                        fs = (lf % 4) * 128
                        nc.tensor.matmul(
                            hp[:, half, 0:w],
                            t_sb[:, 0, lf // 4, fs : fs + 128],
                            m0_sb[:, off : off + w],
                            start=True, stop=False,
                        )
                        nc.tensor.matmul(
                            hp[:, half, 0:w],
                            t_sb[:, jk, lf // 4, fs : fs + 128],
                            m1_sb[:, off : off + w],
                            start=False, stop=True,
                        )
                    hpair = hpool.tile([128, 2, 512], bf16, tag=f"h{pair}")
                    nc.scalar.activation(
                        hpair[:, :, 0:w], hp[:, :, 0:w], AF.Silu, bias=0.0
                    )
                    hs.append(hpair)
                    if fb == 0 and pair == 1:
                        emit_gating_part2a(vi, off, w, jk)
                    if pair >= 1:
                        emit_w2(eop, fb, w, hs[pair - 1], pair - 1)
                if fb == 0:
                    emit_gating_part2b(vi, off, w, jk)
                emit_w2(eop, fb, w, hs[3], 3)
                gp = pmisc.tile([128, w], f32, tag="misc")
                nc.tensor.matmul(
                    gp[:], sel_sb[:, fb, :], gates[vi][:, 0:w],
                    start=True, stop=True,
                )
                gs = gspool.tile([128, 512], bf16, tag="gs")
                nc.vector.tensor_copy(gs[:, 0:w], gp[:])
                if fb == 0:
                    a = apool.tile([128, 512], f32, tag=f"acc{vi}")
                    acc[vi] = a
                    nc.vector.scalar_tensor_tensor(
                        a[:, 0:w], eop[:, 0:w], b2_sb[:, fb : fb + 1],
                        gs[:, 0:w], ALU.add, ALU.mult,
                    )
                else:
                    tmp = tmpool.tile([128, 512], f32, tag="tmp")
                    nc.vector.scalar_tensor_tensor(
                        tmp[:, 0:w], eop[:, 0:w], b2_sb[:, fb : fb + 1],
                        gs[:, 0:w], ALU.add, ALU.mult,
                    )
                    nc.vector.tensor_add(acc[vi][:, 0:w], acc[vi][:, 0:w], tmp[:, 0:w])
                if fb == FB - 1:
                    nc.sync.dma_start(outd[:, off : off + w], acc[vi][:, 0:w])

            # --- expert-phase-major main loop ---
            for fb in range(FB):
                if fb == 0:
                    t0t, t1t = w1t0, w1t1
                else:
                    t0t = w1pool.tile([128, KC, 1024], bf16, tag="w1")
                    for kc in range(KC):
                        nc.sync.dma_start(t0t[:, kc, :], w1d[0, fb, kc])
                    t1t = w1pool.tile([128, KC, 1024], bf16, tag="w1")
                    for kc in range(KC):
                        nc.sync.dma_start(t1t[:, kc, :], w1d[1, fb, kc])
                # build the fb-slice of the 3 chunk tables
                t_sb = tpool.tile([128, 3, 2, 512], bf16, tag="t")

                def emit_tchunk(fb, j, w1t, t_sb):
                    ps = pmm.tile([128, 2, 512], f32, tag="mm")
                    for fp in range(2):
                        for kc in range(KC):
                            nc.tensor.matmul(
                                ps[:, fp, :],
                                embc_sb[:, j, kc, :],
                                w1t[:, kc, fp * 512 : (fp + 1) * 512],
                                start=(kc == 0),
                                stop=(kc == KC - 1 and j != 0),
                            )
                        if j == 0:
                            f0 = fb * 1024 + fp * 512
                            nc.tensor.matmul(
                                ps[:, fp, :], ones128_bf[:],
                                b1r_sb[:, f0 : f0 + 512],
                                start=False, stop=True,
                            )
                    nc.scalar.copy(t_sb[:, j, :, :], ps[:])

                if fb == 0:
                    # chunk 0 first (its W1 block is the first DMA), then the
                    # gating tables fill the PE while w1t1/embc j1,j2 stream
                    emit_tchunk(fb, 0, t0t, t_sb)
                    emit_g(0)
                    emit_tchunk(fb, 1, t1t, t_sb)
                    emit_tchunk(fb, 2, t1t, t_sb)
                    emit_g(1)
                    emit_g(2)
                else:
                    for j in range(3):
                        emit_tchunk(fb, j, t0t if j == 0 else t1t, t_sb)
                for vi, (off, w, jk) in enumerate(visits):
                    emit_visit(fb, vi, off, w, jk, t_sb)

    if legalize:
        _legalize_waits(nc)
    mybir.codegen_inst_isa_subclasses(nc)
    return nc


def _roundup(n, m):
    return -(-n // m) * m


def assign_slots(x):
    """Global (i0//128, i1//128) bucketing: 16 classes -> 8 cores (2 each,
    sharing the i0 chunk), with identical (i0, i1) token pairs DEDUPED —
    the device computes each distinct pair once and the host scatters the
    result to every duplicate token (~11% fewer slots on uniform data)."""
    x = np.asarray(x)
    c0 = x[:, 0] // 128
    c1 = x[:, 1] // 128
    key = x[:, 0] * V + x[:, 1]
    cores = []
    for g in range(NG):
        cls = []
        for b in range(NG):
            idx = np.nonzero((c0 == g) & (c1 == b))[0]
            ukey, inv = np.unique(key[idx], return_inverse=True)
            cls.append((idx, ukey, inv))
        order = sorted(range(NG), key=lambda b: -len(cls[b][1]))
        for pa, pb in ((order[0], order[3]), (order[1], order[2])):
            if len(cls[pb][1]) > len(cls[pa][1]):
                pa, pb = pb, pa
            cores.append(dict(c0=g, c1a=pa, c1b=pb, a=cls[pa], b=cls[pb]))
    SA = _roundup(max(len(c["a"][1]) for c in cores), 64)
    SB = _roundup(max(len(c["b"][1]) for c in cores), 64)
    visits = []
    off = 0
    for span, jk in ((SA, 1), (SB, 2)):
        left = span
        while left > 0:
            w = min(512, left)
            visits.append((off, w, jk))
            off += w
            left -= w
    S = SA + SB
    slot_pairs = []   # per core: slot -> (i0, i1), -1 for pad
    tok_maps = []     # per core: (global token ids, their slots)
    for c in cores:
        si0 = np.full(S, -1, dtype=np.int64)
        si1 = np.full(S, -1, dtype=np.int64)
        (ia, ka, va), (ib, kb, vb) = c["a"], c["b"]
        si0[0 : len(ka)] = ka // V
        si1[0 : len(ka)] = ka % V
        si0[SA : SA + len(kb)] = kb // V
        si1[SA : SA + len(kb)] = kb % V
        slot_pairs.append((si0, si1))
        tok_maps.append(
            (np.concatenate([ia, ib]), np.concatenate([va, SA + vb]))
        )
    return cores, slot_pairs, tok_maps, visits, S


def marshal_inputs(x, emb0, emb1, W1, b1, W2, b2, Wg, bg, cores, slot_pairs, S):
    x = np.asarray(x)
    emb0 = np.asarray(emb0)
    emb1 = np.asarray(emb1)

    shared = {}
    # W1flat[k, f] with f = e*1024 + d (expert-major features)
    w1flat = np.asarray(W1).transpose(1, 0, 2).reshape(IN, F)
    shared["w1m"] = np.ascontiguousarray(
        w1flat.reshape(2, KC, 128, FB, 1024).transpose(0, 3, 1, 2, 4).astype(BF16)
    )
    shared["b1row"] = np.ascontiguousarray(
        np.asarray(b1).reshape(1, F).astype(BF16)
    )
    shared["bgrow"] = np.ascontiguousarray(
        np.asarray(bg).reshape(1, E).astype(BF16)
    )
    shared["wgm"] = np.ascontiguousarray(
        np.asarray(Wg).reshape(2, KC, 128, E).transpose(2, 0, 1, 3).astype(BF16)
    )
    shared["w2s"] = np.ascontiguousarray(
        np.asarray(W2).reshape(E, KC, 128, OUT).transpose(2, 0, 1, 3).astype(BF16)
    )
    shared["b2s"] = np.ascontiguousarray(np.asarray(b2).T.astype(np.float32))
    # sel128[p, e, o] = 1 iff p == e: a (128,128)-tile gate-row broadcast
    sel128 = np.zeros((128, E, 128), dtype=BF16)
    for e in range(E):
        sel128[e, e, :] = 1.0
    shared["sels"] = np.ascontiguousarray(sel128)

    in_maps = []
    for c, (si0, si1) in zip(cores, slot_pairs):
        # embc[p, j, kc, v] = emb_tab(j)[chunk(j)*128 + v, kc*128 + p]
        embc = np.empty((128, 3, KC, 128), dtype=BF16)
        for j, (tab, ch) in enumerate(
            ((emb0, c["c0"]), (emb1, c["c1a"]), (emb1, c["c1b"]))
        ):
            chunk = tab[ch * 128 : (ch + 1) * 128]  # [128v, 1024k]
            embc[:, j] = (
                chunk.reshape(128, KC, 128).transpose(2, 1, 0).astype(BF16)
            )
        # one-hot masks per slot (pad slots stay all-zero)
        m0 = np.zeros((128, S), dtype=BF16)
        m1 = np.zeros((128, S), dtype=BF16)
        pos = np.nonzero(si0 >= 0)[0]
        m0[si0[pos] % 128, pos] = 1.0
        m1[si1[pos] % 128, pos] = 1.0
        in_maps.append(
            {
                "m0": m0,
                "m1": m1,
                "embc": np.ascontiguousarray(embc),
                **shared,
            }
        )
    return in_maps


def kernel(x, emb0, emb1, W1, b1, W2, b2, Wg, bg):
    global LAST_EXEC_NS
    cores, slot_pairs, tok_maps, visits, S = assign_slots(x)
    nc = build_program(visits, S)
    in_maps = marshal_inputs(
        x, emb0, emb1, W1, b1, W2, b2, Wg, bg, cores, slot_pairs, S
    )
    trace = os.environ.get("BASSMOE_TRACE", "0") == "1"
    res = run_bass_kernel_spmd(nc, in_maps, list(range(NCORES)), trace=trace)
    LAST_EXEC_NS = res.exec_time_ns
    out = np.empty((B, OUT), dtype=np.float32)
    for c in range(NCORES):
        toks, slots = tok_maps[c]
        r = res.results[c]["out"]  # [128, S]
        out[toks, :] = r[:, slots].T
    return out
